# revision 11
# baseline (speedup 1.0000x reference)
"""DualReprogrammingLayer Trainium2 kernel.

Sharding: 2 row-groups (B*L split in halves) x 4 head-groups (4 heads each).
Each core computes, for its 2048 rows and 4 heads (per block in {trend, detail}):
  KT = (Wk.T @ protoT)           (heads-slice, S)        [K-proj, f32r in, bf16 out]
  V  = (protoT.T @ Wv) + bv      (S, heads-slice)        [V-proj, f32r]
  qT = (Wq.T @ xT) + bq          (heads-slice, rows)     [bf16]
  scoresT = KT_h @ qT_h          (S, rows) per head      [bf16, 2-head row-packed]
  P  = exp(scoresT / 8)                                  [ACT, f32r out]
  A_ext = [V_h | ones].T @ P     (64+64, rows)           [f32r; rows 64:128 = denom]
  gate = sigmoid(relu(cat @ W1) @ W2)  (on-device, bf16)
  A_scaled = A * (gate_coef / denom)                     [bf16]
  out_partial = [A_t; A_d].T-stack @ [Wo_t; Wo_d]        [bf16]
Host sums the 4 head-group partials per row-group.
"""
import sys
sys.path.insert(0, '/opt/trn_rl_repo')
from contextlib import ExitStack

import numpy as np
import ml_dtypes

import concourse.bass as bass
import concourse.tile as tile
from concourse import bacc, mybir

F32 = mybir.dt.float32
F32R = mybir.dt.float32r
BF16 = mybir.dt.bfloat16
AF = mybir.ActivationFunctionType
bf16 = ml_dtypes.bfloat16

B, L, D, S, DLLM, H, E = 4, 1024, 1024, 1000, 4096, 16, 64
RG, HG = 2, 4                 # row-groups x head-groups = 8 cores
R = (B * L) // RG             # 2048 rows per core
NH = H // HG                  # 4 heads per core
HEC = NH * E                  # 256
SCH, NSC = 125, 8             # S = 8 chunks of 125
RC, NRC = 512, 4              # rows = 4 chunks of 512
KD = D // 128                 # 8 k-chunks for d_model
KL = DLLM // 128              # 32 k-chunks for d_llm

_CACHE = {}
LAST_RESULTS = None           # set by kernel(): BassKernelResults


def _build(with_bo):
    nc = bacc.Bacc("TRN2", target_bir_lowering=False, debug=False)

    def din(name, shape, dt):
        return nc.dram_tensor(name, list(shape), dt, kind="ExternalInput")

    xT = {b: din(f"xT_{b}", (D, R), BF16) for b in "td"}
    pT = {b: din(f"pT_{b}", (DLLM, S), F32) for b in "td"}
    wq = {b: din(f"wq_{b}", (D, HEC), BF16) for b in "td"}
    wk = {b: din(f"wk_{b}", (DLLM, HEC), F32) for b in "td"}
    wv = {b: din(f"wv_{b}", (DLLM, HEC), F32) for b in "td"}
    wo = din("wo", (2 * HEC, DLLM), BF16)            # [t rows | d rows]
    w1 = din("w1", (2 * D, D), BF16)
    w2 = din("w2", (D, 1), BF16)
    bq2 = din("bq2", (128, 4), F32)                  # cols: t-mc0, t-mc1, d-mc0, d-mc1
    bk2 = din("bk2", (128, 4), F32)
    bvv = din("bv", (1, 2 * HEC), F32)               # [t 256 | d 256]
    gb1 = din("gb1", (128, KD), F32)
    gb2 = din("gb2", (1, 1), F32)
    ones_d = din("ones", (1, 2048), F32)
    bo2 = din("bo2", (2, DLLM), BF16) if with_bo else None
    out = nc.dram_tensor("out", [R, DLLM], F32, kind="ExternalOutput")

    with tile.TileContext(nc) as tc, ExitStack() as ctx:
        # ---- persistent pools (live across phases) ----
        pers = ctx.enter_context(tc.tile_pool(name="pers", bufs=1))
        kt_sb = {}    # block -> tile (128, 2, S) bf16 : HE chunk mc at [:, mc, :]
        vx_sb = {}    # block -> tile (125, NSC, NH, 128) f32r : [V_h | ones]
        qt_sb = {}    # block -> tile (128, 2, R) bf16
        for b in "td":
            kt_sb[b] = pers.tile([128, 2, S], BF16, tag=f"kt_{b}", name=f"kt_{b}")
            vx_sb[b] = pers.tile([SCH, NSC, NH, 128], F32R, tag=f"vx_{b}", name=f"vx_{b}")
            qt_sb[b] = pers.tile([128, 2, R], BF16, tag=f"qt_{b}", name=f"qt_{b}")
        gate_sb = pers.tile([1, R], F32, tag="gate")     # sigmoid output
        omg_sb = pers.tile([1, R], F32, tag="omg")       # 1 - gate
        ones125 = pers.tile([1, SCH], F32R, tag="ones125")
        nc.sync.dma_start(ones125[:], ones_d.ap()[0:1, 0:SCH].bitcast(F32R))
        onesrow = pers.tile([1, RC], F32, tag="onesrow")
        nc.vector.memset(onesrow[:], 1.0)
        bq_sb = pers.tile([128, 4], F32, tag="bq")
        nc.sync.dma_start(bq_sb[:], bq2.ap())
        bk_sb = pers.tile([128, 4], F32, tag="bk")
        nc.sync.dma_start(bk_sb[:], bk2.ap())
        bv_sb = pers.tile([1, 2 * HEC], F32R, tag="bv")
        nc.sync.dma_start(bv_sb[:], bvv.ap().bitcast(F32R))
        gb1_sb = pers.tile([128, KD], F32, tag="gb1")
        nc.sync.dma_start(gb1_sb[:], gb1.ap())
        gb2_sb = pers.tile([1, 1], F32, tag="gb2")
        nc.sync.dma_start(gb2_sb[:], gb2.ap())
        if with_bo:
            bo_sb = pers.tile([2, DLLM], BF16, tag="bo")
            nc.sync.dma_start(bo_sb[:], bo2.ap())
            g2_sb = pers.tile([2, R], BF16, tag="g2")

        # ---- phase A: K/V projections ----
        with ExitStack() as actx:
            p_w = actx.enter_context(tc.tile_pool(name="p_w", bufs=1))
            p_pt = actx.enter_context(tc.tile_pool(name="p_pt", bufs=8))
            psA = actx.enter_context(tc.tile_pool(name="psA", bufs=1, space="PSUM"))
            for b in "td":
                wk_t = p_w.tile([128, KL, HEC], F32R, tag="wk")
                nc.sync.dma_start(
                    wk_t[:], wk[b].ap().rearrange("(c p) m -> p c m", p=128).bitcast(F32R))
                wv_t = p_w.tile([128, KL, HEC], F32R, tag="wv")
                nc.sync.dma_start(
                    wv_t[:], wv[b].ap().rearrange("(c p) m -> p c m", p=128).bitcast(F32R))

                vps = [psA.tile([SCH, 2, HEC], F32, tag=f"vps{i}", name=f"vps{i}")
                       for i in range(4)]
                kps = [psA.tile([128, 512], F32, tag=f"kps{i}", name=f"kps{i}")
                       for i in range(4)]
                for kc in range(KL):
                    pt_t = p_pt.tile([128, S], F32R, tag="pt")
                    nc.sync.dma_start(
                        pt_t[:],
                        pT[b].ap().rearrange("(c p) s -> c p s", c=KL)[kc].bitcast(F32R))
                    for si in range(NSC):
                        # one accumulation group per PSUM bank: only the first
                        # half issues start=True (bank-wide clear covers both)
                        nc.tensor.matmul(
                            vps[si // 2][:, si % 2, :],
                            pt_t[:, si * SCH:(si + 1) * SCH],
                            wv_t[:, kc, :],
                            start=(kc == 0 and si % 2 == 0), stop=False)
                    for mc in range(2):
                        for ncc in range(2):
                            nc.tensor.matmul(
                                kps[mc * 2 + ncc][:, 0:500],
                                wk_t[:, kc, mc * 128:(mc + 1) * 128],
                                pt_t[:, ncc * 500:(ncc + 1) * 500],
                                start=(kc == 0), stop=(kc == KL - 1))
                boff = 0 if b == "t" else HEC
                for si in range(NSC):
                    nc.tensor.matmul(
                        vps[si // 2][:, si % 2, :],
                        ones125[:],
                        bv_sb[:, boff:boff + HEC],
                        start=False, stop=(si % 2 == 1))
                for si in range(NSC):
                    # copy V psum (125, 256) -> [:, si, :, 0:64] viewed as (125, 4, 64)
                    nc.vector.tensor_copy(
                        vx_sb[b][:, si, :, 0:64],
                        vps[si // 2][:, si % 2, :].rearrange("p (h e) -> p h e", h=NH))
                ones_bc = bass.AP(tensor=ones_d.ap().tensor, offset=0,
                                  ap=[[0, SCH], [NH * 64, NSC], [64, NH], [1, 64]]
                                  ).bitcast(F32R)
                nc.sync.dma_start(vx_sb[b][:, :, :, 64:128], ones_bc)
                for mc in range(2):
                    for ncc in range(2):
                        nc.scalar.activation(
                            kt_sb[b][:, mc, ncc * 500:(ncc + 1) * 500],
                            kps[mc * 2 + ncc][:, 0:500],
                            AF.Identity,
                            bias=bk_sb[:, (0 if b == "t" else 2) + mc:
                                       (0 if b == "t" else 2) + mc + 1])

        # ---- phase B: gate + Q projections (per rows-chunk) ----
        with ExitStack() as bctx:
            p_w1 = bctx.enter_context(tc.tile_pool(name="p_w1", bufs=1))
            p_x = bctx.enter_context(tc.tile_pool(name="p_x", bufs=3))
            p_h = bctx.enter_context(tc.tile_pool(name="p_h", bufs=2))
            psB = bctx.enter_context(tc.tile_pool(name="psB", bufs=2, space="PSUM"))
            w1_t = p_w1.tile([128, 2 * KD, D], BF16, tag="w1")
            nc.sync.dma_start(w1_t[:], w1.ap().rearrange("(c p) m -> p c m", p=128))
            w2_t = p_w1.tile([128, KD, 1], BF16, tag="w2")
            nc.sync.dma_start(w2_t[:], w2.ap().rearrange("(c p) m -> p c m", p=128))
            wq_t = {}
            for b in "td":
                wq_t[b] = p_w1.tile([128, KD, HEC], BF16, tag=f"wq_{b}", name=f"wq_{b}")
                nc.sync.dma_start(
                    wq_t[b][:], wq[b].ap().rearrange("(c p) m -> p c m", p=128))

            for r in range(NRC):
                rsl = slice(r * RC, (r + 1) * RC)
                xt = {}
                for b in "td":
                    xt[b] = p_x.tile([128, KD, RC], BF16, tag=f"x_{b}", name=f"x_{b}")
                    nc.sync.dma_start(
                        xt[b][:],
                        xT[b].ap().rearrange("(c p) n -> p c n", p=128)[:, :, rsl])
                # gate hidden: 8 m-chunks, contraction over 16 chunks (t then d)
                ht = p_h.tile([128, KD, RC], BF16, tag="ht")
                for mc in range(KD):
                    hps = psB.tile([128, RC], F32, tag="hps")
                    for kc in range(2 * KD):
                        nc.tensor.matmul(
                            hps[:],
                            w1_t[:, kc, mc * 128:(mc + 1) * 128],
                            xt["t" if kc < KD else "d"][:, kc % KD, :],
                            start=(kc == 0), stop=(kc == 2 * KD - 1))
                    nc.scalar.activation(
                        ht[:, mc, :], hps[:], AF.Relu,
                        bias=gb1_sb[:, mc:mc + 1])
                lps = psB.tile([1, RC], F32, tag="lps")
                for mc in range(KD):
                    nc.tensor.matmul(
                        lps[:], w2_t[:, mc, :], ht[:, mc, :],
                        start=(mc == 0), stop=(mc == KD - 1))
                nc.scalar.activation(
                    gate_sb[:, rsl], lps[:], AF.Sigmoid, bias=gb2_sb[:])
                nc.vector.tensor_sub(omg_sb[:, rsl], onesrow[:], gate_sb[:, rsl])
                if with_bo:
                    nc.vector.tensor_copy(g2_sb[0:1, rsl], gate_sb[:, rsl])
                    nc.vector.tensor_copy(g2_sb[1:2, rsl], omg_sb[:, rsl])
                # Q projections
                for b in "td":
                    for mc in range(2):
                        qps = psB.tile([128, RC], F32, tag="qps")
                        for kc in range(KD):
                            nc.tensor.matmul(
                                qps[:],
                                wq_t[b][:, kc, mc * 128:(mc + 1) * 128],
                                xt[b][:, kc, :],
                                start=(kc == 0), stop=(kc == KD - 1))
                        nc.scalar.activation(
                            qt_sb[b][:, mc, rsl], qps[:], AF.Identity,
                            bias=bq_sb[:, (0 if b == "t" else 2) + mc:
                                       (0 if b == "t" else 2) + mc + 1])

        # ---- phase C: attention + output projection (per rows-chunk) ----
        with ExitStack() as cctx:
            p_wo = cctx.enter_context(tc.tile_pool(name="p_wo", bufs=1))
            p_p = cctx.enter_context(tc.tile_pool(name="p_p", bufs=3))
            p_a = cctx.enter_context(tc.tile_pool(name="p_a", bufs=2))
            p_s = cctx.enter_context(tc.tile_pool(name="p_s", bufs=2))
            p_o = cctx.enter_context(tc.tile_pool(name="p_o", bufs=4))
            psS = cctx.enter_context(tc.tile_pool(name="psS", bufs=2, space="PSUM"))
            psPV = cctx.enter_context(tc.tile_pool(name="psPV", bufs=1, space="PSUM"))
            psO = cctx.enter_context(tc.tile_pool(name="psO", bufs=2, space="PSUM"))

            wo_t = p_wo.tile([128, 4, DLLM], BF16, tag="wo")
            nc.sync.dma_start(wo_t[:], wo.ap().rearrange("(c p) n -> p c n", p=128))

            for r in range(NRC):
                rsl = slice(r * RC, (r + 1) * RC)
                # a2[b][mc]: (128, RC) bf16 for this rows-chunk (2 head-pairs stacked)
                a2 = {b: [p_a.tile([128, RC], BF16, tag=f"a2_{b}{mc}", bufs=2,
                                   name=f"a2_{b}{mc}")
                          for mc in range(2)] for b in "td"}
                for b in "td":
                    gcoef = gate_sb if b == "t" else omg_sb
                    gbc = p_s.tile([64, RC], F32, tag="gbc")
                    nc.gpsimd.partition_broadcast(gbc[:], gcoef[:, rsl])
                    for mc in range(2):  # head pair
                        aps = {}
                        for hh in range(2):
                            aps[hh] = psPV.tile([128, RC], F32, tag=f"aps{hh}",
                                                name=f"aps{hh}")
                        for si in range(NSC):
                            pts = {}
                            for hh in range(2):  # row-packed pair, adjacent emission
                                po = hh * 64
                                sps = psS.tile([SCH, RC], F32, tag=f"sps{hh}",
                                               name=f"sps{hh}")
                                nc.tensor.matmul(
                                    sps[:],
                                    kt_sb[b][po:po + 64, mc, si * SCH:(si + 1) * SCH],
                                    qt_sb[b][po:po + 64, mc, rsl],
                                    start=True, stop=True, tile_position=(po, 0))
                                pt = p_p.tile([SCH, RC], F32R, tag=f"p{hh}",
                                              name=f"p{hh}")
                                nc.scalar.activation(pt[:], sps[:], AF.Exp, scale=0.125)
                                pts[hh] = pt
                            for hh in range(2):
                                h = mc * 2 + hh
                                nc.tensor.matmul(
                                    aps[hh][:], vx_sb[b][:, si, h, :], pts[hh][:],
                                    start=(si == 0), stop=(si == NSC - 1))
                        for hh in range(2):
                            po = hh * 64
                            rec = p_s.tile([64, RC], F32, tag="rec")
                            nc.vector.reciprocal(rec[:], aps[hh][64:128, :])
                            sct = p_s.tile([64, RC], F32, tag="sct")
                            nc.vector.tensor_mul(sct[:], rec[:], gbc[:])
                            nc.vector.tensor_mul(
                                a2[b][mc][po:po + 64, :], aps[hh][0:64, :], sct[:])
                # output projection for this rows-chunk
                for rb in range(4):
                    row0 = r * RC + rb * 128
                    for ncc in range(8):
                        nsl = slice(ncc * 512, (ncc + 1) * 512)
                        ops = psO.tile([128, 512], F32, tag="ops")
                        chains = [("t", 0), ("t", 1), ("d", 0), ("d", 1)]
                        for kk, (bb, mc) in enumerate(chains):
                            nc.tensor.matmul(
                                ops[:], a2[bb][mc][:, rb * 128:(rb + 1) * 128],
                                wo_t[:, kk, nsl],
                                start=(kk == 0), stop=(kk == 3 and not with_bo))
                        if with_bo:
                            nc.tensor.matmul(
                                ops[:], g2_sb[:, row0:row0 + 128], bo_sb[:, nsl],
                                start=False, stop=True)
                        osb = p_o.tile([128, 512], F32, tag="osb")
                        nc.vector.tensor_copy(osb[:], ops[:])
                        nc.sync.dma_start(out.ap()[row0:row0 + 128, nsl], osb[:])

    nc.compile()
    return nc


def _prep_inputs(inputs):
    """Host-side shard + transpose. Returns in_maps for 8 cores."""
    f32 = np.float32
    t = {k: np.asarray(v) for k, v in inputs.items()}
    x_full = {"t": t["trend_emb"].reshape(B * L, D).astype(f32),
              "d": t["detail_emb"].reshape(B * L, D).astype(f32)}
    pT_full = {"t": np.ascontiguousarray(t["trend_proto"].astype(f32).T),
               "d": np.ascontiguousarray(t["detail_proto"].astype(f32).T)}
    W = {("q", "t"): t["t_Wq"], ("q", "d"): t["d_Wq"],
         ("k", "t"): t["t_Wk"], ("k", "d"): t["d_Wk"],
         ("v", "t"): t["t_Wv"], ("v", "d"): t["d_Wv"],
         ("o", "t"): t["t_Wo"], ("o", "d"): t["d_Wo"]}
    bias = {("q", "t"): t["t_bq"], ("q", "d"): t["d_bq"],
            ("k", "t"): t["t_bk"], ("k", "d"): t["d_bk"],
            ("v", "t"): t["t_bv"], ("v", "d"): t["d_bv"],
            ("o", "t"): t["t_bo"], ("o", "d"): t["d_bo"]}

    with_bo = bool(np.any(bias[("o", "t")]) or np.any(bias[("o", "d")]))
    in_maps = []
    for core in range(8):
        rg, hg = divmod(core, HG)
        rows = slice(rg * R, (rg + 1) * R)
        hsl = slice(hg * HEC, (hg + 1) * HEC)
        m = {}
        for b in "td":
            m[f"xT_{b}"] = np.ascontiguousarray(x_full[b][rows].T).astype(bf16)
            m[f"pT_{b}"] = pT_full[b]
            m[f"wq_{b}"] = np.ascontiguousarray(W[("q", b)][:, hsl]).astype(bf16)
            m[f"wk_{b}"] = np.ascontiguousarray(W[("k", b)][:, hsl]).astype(f32)
            m[f"wv_{b}"] = np.ascontiguousarray(W[("v", b)][:, hsl]).astype(f32)
        m["wo"] = np.vstack([W[("o", "t")][hsl, :], W[("o", "d")][hsl, :]]).astype(bf16)
        m["w1"] = t["g_W1"].astype(bf16)
        m["w2"] = t["g_W2"].astype(bf16)
        m["bq2"] = np.stack([bias[("q", "t")][hsl][0:128], bias[("q", "t")][hsl][128:256],
                             bias[("q", "d")][hsl][0:128], bias[("q", "d")][hsl][128:256]],
                            axis=1).astype(f32)
        m["bk2"] = np.stack([bias[("k", "t")][hsl][0:128], bias[("k", "t")][hsl][128:256],
                             bias[("k", "d")][hsl][0:128], bias[("k", "d")][hsl][128:256]],
                            axis=1).astype(f32)
        m["bv"] = np.concatenate([bias[("v", "t")][hsl],
                                  bias[("v", "d")][hsl]])[None, :].astype(f32)
        m["gb1"] = np.ascontiguousarray(
            t["g_b1"].astype(f32).reshape(KD, 128).T)
        m["gb2"] = t["g_b2"].astype(f32).reshape(1, 1)
        m["ones"] = np.ones((1, 2048), f32)
        if with_bo:
            m["bo2"] = (np.stack([bias[("o", "t")], bias[("o", "d")]]) / HG).astype(bf16)
        in_maps.append(m)
    return in_maps, with_bo


def kernel(**inputs):
    global LAST_RESULTS
    import os
    from concourse.bass_utils import run_bass_kernel_spmd

    in_maps, with_bo = _prep_inputs(inputs)
    if with_bo not in _CACHE:
        _CACHE[with_bo] = _build(with_bo)
    nc = _CACHE[with_bo]

    trace = bool(os.environ.get("KERNEL_TRACE"))
    res = run_bass_kernel_spmd(
        nc, in_maps, list(range(8)),
        trace=trace, trace_cores=list(range(8)) if trace else None)
    LAST_RESULTS = res

    out = np.empty((RG, R, DLLM), np.float32)
    for rg in range(RG):
        acc = res.results[rg * HG]["out"].astype(np.float32)
        for hg in range(1, HG):
            acc = acc + res.results[rg * HG + hg]["out"]
        out[rg] = acc
    return out.reshape(B, L, DLLM)


# revision 18
# speedup vs baseline: 1.0663x; 1.0663x over previous
"""DualReprogrammingLayer Trainium2 kernel.

Sharding: 2 row-groups (B*L split in halves) x 4 head-groups (4 heads each).
Each core computes, for its 2048 rows and 4 heads (per block in {trend, detail}):
  KT = (Wk.T @ protoT)           (heads-slice, S)        [K-proj, f32r in, bf16 out]
  V  = (protoT.T @ Wv) + bv      (S, heads-slice)        [V-proj, f32r]
  qT = (Wq.T @ xT) + bq          (heads-slice, rows)     [bf16]
  scoresT = KT_h @ qT_h          (S, rows) per head      [bf16, 2-head row-packed]
  P  = exp(scoresT / 8)                                  [ACT, f32r out]
  A_ext = [V_h | ones].T @ P     (64+64, rows)           [f32r; rows 64:128 = denom]
  gate = sigmoid(relu(cat @ W1) @ W2)  (on-device, bf16)
  A_scaled = A * (gate_coef / denom)                     [bf16]
  out_partial = [A_t; A_d].T-stack @ [Wo_t; Wo_d]        [bf16]
Host sums the 4 head-group partials per row-group.
"""
import sys
sys.path.insert(0, '/opt/trn_rl_repo')
from contextlib import ExitStack

import numpy as np
import ml_dtypes

import concourse.bass as bass
import concourse.tile as tile
from concourse import bacc, mybir

F32 = mybir.dt.float32
F32R = mybir.dt.float32r
BF16 = mybir.dt.bfloat16
AF = mybir.ActivationFunctionType
bf16 = ml_dtypes.bfloat16

B, L, D, S, DLLM, H, E = 4, 1024, 1024, 1000, 4096, 16, 64
RG, HG = 2, 4                 # row-groups x head-groups = 8 cores
R = (B * L) // RG             # 2048 rows per core
NH = H // HG                  # 4 heads per core
HEC = NH * E                  # 256
SCH, NSC = 125, 8             # S = 8 chunks of 125
RC, NRC = 512, 4              # rows = 4 chunks of 512
KD = D // 128                 # 8 k-chunks for d_model
KL = DLLM // 128              # 32 k-chunks for d_llm

_CACHE = {}
LAST_RESULTS = None           # set by kernel(): BassKernelResults


def _build(with_bo):
    nc = bacc.Bacc("TRN2", target_bir_lowering=False, debug=False)

    def din(name, shape, dt):
        return nc.dram_tensor(name, list(shape), dt, kind="ExternalInput")

    xT = {b: din(f"xT_{b}", (D, R), BF16) for b in "td"}
    pT = {b: din(f"pT_{b}", (DLLM, S), F32) for b in "td"}
    wq = {b: din(f"wq_{b}", (D, HEC), BF16) for b in "td"}
    wk = {b: din(f"wk_{b}", (DLLM, HEC), F32) for b in "td"}
    wv = {b: din(f"wv_{b}", (DLLM, HEC), F32) for b in "td"}
    wo = din("wo", (2 * HEC, DLLM), BF16)            # [t rows | d rows]
    w1 = din("w1", (2 * D, D), BF16)
    w2 = din("w2", (D, 1), BF16)
    bq2 = din("bq2", (128, 4), F32)                  # cols: t-mc0, t-mc1, d-mc0, d-mc1
    bk2 = din("bk2", (128, 4), F32)
    bvv = din("bv", (1, 2 * HEC), F32)               # [t 256 | d 256]
    gb1 = din("gb1", (128, KD), F32)
    gb2 = din("gb2", (1, 1), F32)
    ones_d = din("ones", (1, 2048), F32)
    bo2 = din("bo2", (2, DLLM), BF16) if with_bo else None
    out = nc.dram_tensor("out", [R, DLLM], F32, kind="ExternalOutput")

    with tile.TileContext(nc) as tc, ExitStack() as ctx:
        # ---- persistent pools (live across phases) ----
        pers = ctx.enter_context(tc.tile_pool(name="pers", bufs=1))
        kt_sb = {}    # block -> tile (128, 2, S) bf16 : HE chunk mc at [:, mc, :]
        vx_sb = {}    # block -> tile (125, NSC, NH, 128) f32r : [V_h | ones]
        qt_sb = {}    # block -> tile (128, 2, R) bf16
        for b in "td":
            kt_sb[b] = pers.tile([128, 2, S], BF16, tag=f"kt_{b}", name=f"kt_{b}")
            vx_sb[b] = pers.tile([SCH, NSC, NH, 65], F32R, tag=f"vx_{b}", name=f"vx_{b}")
            qt_sb[b] = pers.tile([128, 2, R], BF16, tag=f"qt_{b}", name=f"qt_{b}")
        gate_sb = pers.tile([1, R], F32, tag="gate")     # sigmoid output
        omg_sb = pers.tile([1, R], F32, tag="omg")       # 1 - gate
        ones125 = pers.tile([1, SCH], F32R, tag="ones125")
        nc.sync.dma_start(ones125[:], ones_d.ap()[0:1, 0:SCH].bitcast(F32R))
        onesrow = pers.tile([1, RC], F32, tag="onesrow")
        nc.vector.memset(onesrow[:], 1.0)
        bq_sb = pers.tile([128, 4], F32, tag="bq")
        nc.sync.dma_start(bq_sb[:], bq2.ap())
        bk_sb = pers.tile([128, 4], F32, tag="bk")
        nc.sync.dma_start(bk_sb[:], bk2.ap())
        bv_sb = pers.tile([1, 2 * HEC], F32R, tag="bv")
        nc.sync.dma_start(bv_sb[:], bvv.ap().bitcast(F32R))
        gb1_sb = pers.tile([128, KD], F32, tag="gb1")
        nc.sync.dma_start(gb1_sb[:], gb1.ap())
        gb2_sb = pers.tile([1, 1], F32, tag="gb2")
        nc.sync.dma_start(gb2_sb[:], gb2.ap())
        if with_bo:
            bo_sb = pers.tile([2, DLLM], BF16, tag="bo")
            nc.sync.dma_start(bo_sb[:], bo2.ap())
            g2_sb = pers.tile([2, R], BF16, tag="g2")

        # ---- prefetch pools: weights for later phases, loaded during phase A.
        # p_pre2 (wo) lives through phase C; p_pre1 (W1/wq) releases after B.
        p_pre2 = ctx.enter_context(tc.tile_pool(name="p_pre2", bufs=1))
        wo_t = p_pre2.tile([128, 4, DLLM], BF16, tag="wo")
        nc.sync.dma_start(wo_t[:], wo.ap().rearrange("(c p) n -> p c n", p=128))
        pre1ctx = ExitStack()
        p_pre1 = pre1ctx.enter_context(tc.tile_pool(name="p_pre1", bufs=1))
        w1_t = p_pre1.tile([128, 2 * KD, D], BF16, tag="w1")
        nc.sync.dma_start(w1_t[:], w1.ap().rearrange("(c p) m -> p c m", p=128))
        w2_t = p_pre1.tile([128, KD, 1], BF16, tag="w2")
        nc.sync.dma_start(w2_t[:], w2.ap().rearrange("(c p) m -> p c m", p=128))
        wq_t = {}
        for b in "td":
            wq_t[b] = p_pre1.tile([128, KD, HEC], BF16, tag=f"wq_{b}", name=f"wq_{b}")
            nc.sync.dma_start(
                wq_t[b][:], wq[b].ap().rearrange("(c p) m -> p c m", p=128))

        # ---- phase A: K/V projections (proto and weights streamed per k-chunk) ----
        with ExitStack() as actx:
            p_pt = actx.enter_context(tc.tile_pool(name="p_pt", bufs=6))
            p_wc = actx.enter_context(tc.tile_pool(name="p_wc", bufs=6))
            psA = actx.enter_context(tc.tile_pool(name="psA", bufs=1, space="PSUM"))
            for b in "td":
                vps = [psA.tile([SCH, 2, HEC], F32, tag=f"vps{i}", name=f"vps{i}")
                       for i in range(4)]
                kps = [psA.tile([128, 512], F32, tag=f"kps{i}", name=f"kps{i}")
                       for i in range(4)]
                wk_r = wk[b].ap().rearrange("(c p) m -> c p m", c=KL).bitcast(F32R)
                wv_r = wv[b].ap().rearrange("(c p) m -> c p m", c=KL).bitcast(F32R)
                pt_r = pT[b].ap().rearrange("(c p) s -> c p s", c=KL).bitcast(F32R)
                for kc in range(KL):
                    pt_t = p_pt.tile([128, S], F32R, tag="pt")
                    nc.sync.dma_start(pt_t[:], pt_r[kc])
                    wkc = p_wc.tile([128, HEC], F32R, tag="wkc")
                    nc.sync.dma_start(wkc[:], wk_r[kc])
                    wvc = p_wc.tile([128, HEC], F32R, tag="wvc")
                    nc.sync.dma_start(wvc[:], wv_r[kc])
                    for si in range(NSC):
                        # one accumulation group per PSUM bank: only the first
                        # half issues start=True (bank-wide clear covers both)
                        nc.tensor.matmul(
                            vps[si // 2][:, si % 2, :],
                            pt_t[:, si * SCH:(si + 1) * SCH],
                            wvc[:],
                            start=(kc == 0 and si % 2 == 0), stop=False)
                    for mc in range(2):
                        for ncc in range(2):
                            nc.tensor.matmul(
                                kps[mc * 2 + ncc][:, 0:500],
                                wkc[:, mc * 128:(mc + 1) * 128],
                                pt_t[:, ncc * 500:(ncc + 1) * 500],
                                start=(kc == 0), stop=(kc == KL - 1))
                boff = 0 if b == "t" else HEC
                for si in range(NSC):
                    nc.tensor.matmul(
                        vps[si // 2][:, si % 2, :],
                        ones125[:],
                        bv_sb[:, boff:boff + HEC],
                        start=False, stop=(si % 2 == 1))
                for si in range(NSC):
                    # copy V psum (125, 256) -> [:, si, :, 0:64] viewed as (125, 4, 64)
                    nc.vector.tensor_copy(
                        vx_sb[b][:, si, :, 0:64],
                        vps[si // 2][:, si % 2, :].rearrange("p (h e) -> p h e", h=NH))
                ones_bc = bass.AP(tensor=ones_d.ap().tensor, offset=0,
                                  ap=[[0, SCH], [1, NH], [1, 1]]).bitcast(F32R)
                for si in range(NSC):
                    nc.sync.dma_start(vx_sb[b][:, si, :, 64:65], ones_bc)
                for mc in range(2):
                    for ncc in range(2):
                        nc.scalar.activation(
                            kt_sb[b][:, mc, ncc * 500:(ncc + 1) * 500],
                            kps[mc * 2 + ncc][:, 0:500],
                            AF.Identity,
                            bias=bk_sb[:, (0 if b == "t" else 2) + mc:
                                       (0 if b == "t" else 2) + mc + 1])

        # ---- phase B: gate + Q projections (per rows-chunk) ----
        with ExitStack() as bctx:
            p_x = bctx.enter_context(tc.tile_pool(name="p_x", bufs=3))
            p_h = bctx.enter_context(tc.tile_pool(name="p_h", bufs=2))
            psB = bctx.enter_context(tc.tile_pool(name="psB", bufs=2, space="PSUM"))
            for r in range(NRC):
                rsl = slice(r * RC, (r + 1) * RC)
                xt = {}
                for b in "td":
                    xt[b] = p_x.tile([128, KD, RC], BF16, tag=f"x_{b}", name=f"x_{b}")
                    nc.sync.dma_start(
                        xt[b][:],
                        xT[b].ap().rearrange("(c p) n -> p c n", p=128)[:, :, rsl])
                # gate hidden: 8 m-chunks, contraction over 16 chunks (t then d)
                ht = p_h.tile([128, KD, RC], BF16, tag="ht")
                for mc in range(KD):
                    hps = psB.tile([128, RC], F32, tag="hps")
                    for kc in range(2 * KD):
                        nc.tensor.matmul(
                            hps[:],
                            w1_t[:, kc, mc * 128:(mc + 1) * 128],
                            xt["t" if kc < KD else "d"][:, kc % KD, :],
                            start=(kc == 0), stop=(kc == 2 * KD - 1))
                    nc.scalar.activation(
                        ht[:, mc, :], hps[:], AF.Relu,
                        bias=gb1_sb[:, mc:mc + 1])
                lps = psB.tile([1, RC], F32, tag="lps")
                for mc in range(KD):
                    nc.tensor.matmul(
                        lps[:], w2_t[:, mc, :], ht[:, mc, :],
                        start=(mc == 0), stop=(mc == KD - 1))
                nc.scalar.activation(
                    gate_sb[:, rsl], lps[:], AF.Sigmoid, bias=gb2_sb[:])
                nc.vector.tensor_sub(omg_sb[:, rsl], onesrow[:], gate_sb[:, rsl])
                if with_bo:
                    nc.vector.tensor_copy(g2_sb[0:1, rsl], gate_sb[:, rsl])
                    nc.vector.tensor_copy(g2_sb[1:2, rsl], omg_sb[:, rsl])
                # Q projections
                for b in "td":
                    for mc in range(2):
                        qps = psB.tile([128, RC], F32, tag="qps")
                        for kc in range(KD):
                            nc.tensor.matmul(
                                qps[:],
                                wq_t[b][:, kc, mc * 128:(mc + 1) * 128],
                                xt[b][:, kc, :],
                                start=(kc == 0), stop=(kc == KD - 1))
                        nc.scalar.activation(
                            qt_sb[b][:, mc, rsl], qps[:], AF.Identity,
                            bias=bq_sb[:, (0 if b == "t" else 2) + mc:
                                       (0 if b == "t" else 2) + mc + 1])

        pre1ctx.close()

        # ---- phase C: attention + output projection (per rows-chunk) ----
        # Software pipeline over the 4 (block, head-pair) units per rows-chunk:
        # at s-chunk granularity, QK+exp of unit u interleaves with PV of u-1,
        # so the PE never sits idle waiting for ACT's exp.
        with ExitStack() as cctx:
            p_p = cctx.enter_context(tc.tile_pool(name="p_p", bufs=6))
            p_a = cctx.enter_context(tc.tile_pool(name="p_a", bufs=2))
            p_s = cctx.enter_context(tc.tile_pool(name="p_s", bufs=3))
            p_o = cctx.enter_context(tc.tile_pool(name="p_o", bufs=4))
            psS = cctx.enter_context(tc.tile_pool(name="psS", bufs=1, space="PSUM"))
            psPV = cctx.enter_context(tc.tile_pool(name="psPV", bufs=1, space="PSUM"))
            psO = cctx.enter_context(tc.tile_pool(name="psO", bufs=2, space="PSUM"))

            def emit_qk_exp(b, mc, si, rsl):
                sps2 = psS.tile([SCH, 2, RC], F32, tag="sps", name="sps")
                for hh in range(2):  # row-packed pair, adjacent emission
                    po = hh * 64
                    nc.tensor.matmul(
                        sps2[:, hh, :],
                        kt_sb[b][po:po + 64, mc, si * SCH:(si + 1) * SCH],
                        qt_sb[b][po:po + 64, mc, rsl],
                        start=True, stop=True,
                        tile_position=(po, 0))
                p2 = p_p.tile([SCH, 2, RC], F32R, tag=f"p{si % 2}",
                              name=f"p{si % 2}")
                nc.scalar.activation(p2[:], sps2[:], AF.Exp, scale=0.125)
                return p2

            def emit_pv(aps, b, mc, si, p2):
                for hh in range(2):
                    h = mc * 2 + hh
                    nc.tensor.matmul(
                        aps[hh][:], vx_sb[b][:, si, h, :], p2[:, hh, :],
                        start=(si == 0), stop=(si == NSC - 1))

            def emit_norm(aps, b, mc, a2, rsl):
                gcoef = gate_sb if b == "t" else omg_sb
                for hh in range(2):
                    rec1 = p_s.tile([1, RC], F32, tag="rec1")
                    nc.vector.reciprocal(rec1[:], aps[hh][64:65, :])
                    sct1 = p_s.tile([1, RC], F32, tag="sct1")
                    nc.vector.tensor_mul(sct1[:], rec1[:], gcoef[:, rsl])
                    sct64 = p_s.tile([64, RC], F32, tag="sct64")
                    nc.gpsimd.partition_broadcast(sct64[:], sct1[:])
                    nc.vector.tensor_mul(
                        a2[b][mc][hh * 64:hh * 64 + 64, :],
                        aps[hh][0:64, :], sct64[:])

            for r in range(NRC):
                rsl = slice(r * RC, (r + 1) * RC)
                a2 = {b: [p_a.tile([128, RC], BF16, tag=f"a2_{b}{mc}",
                                   name=f"a2_{b}{mc}")
                          for mc in range(2)] for b in "td"}
                units = [(b, mc) for b in "td" for mc in range(2)]
                prev = None   # (aps, b, mc, p2list)
                for b, mc in units:
                    aps = [psPV.tile([65, RC], F32, tag=f"aps{mc}{hh}",
                                     name=f"aps{mc}{hh}") for hh in range(2)]
                    p2buf = {}
                    for si in range(NSC):
                        p2buf[si] = emit_qk_exp(b, mc, si, rsl)
                        if prev is not None:
                            paps, pb, pmc, pp2 = prev
                            emit_pv(paps, pb, pmc, si, pp2[si])
                            pp2[si] = None
                    if prev is not None:
                        emit_norm(prev[0], prev[1], prev[2], a2p, rsl_p)
                    prev = (aps, b, mc, p2buf)
                    a2p, rsl_p = a2, rsl
                # drain last unit of this rows-chunk
                paps, pb, pmc, pp2 = prev
                for si in range(NSC):
                    emit_pv(paps, pb, pmc, si, pp2[si])
                emit_norm(paps, pb, pmc, a2, rsl)

                # output projection for this rows-chunk
                for rb in range(4):
                    row0 = r * RC + rb * 128
                    for ncc in range(8):
                        nsl = slice(ncc * 512, (ncc + 1) * 512)
                        ops = psO.tile([128, 512], F32, tag="ops")
                        chains = [("t", 0), ("t", 1), ("d", 0), ("d", 1)]
                        for kk, (bb, mcc) in enumerate(chains):
                            nc.tensor.matmul(
                                ops[:], a2[bb][mcc][:, rb * 128:(rb + 1) * 128],
                                wo_t[:, kk, nsl],
                                start=(kk == 0), stop=(kk == 3 and not with_bo))
                        if with_bo:
                            nc.tensor.matmul(
                                ops[:], g2_sb[:, row0:row0 + 128], bo_sb[:, nsl],
                                start=False, stop=True)
                        osb = p_o.tile([128, 512], F32, tag="osb")
                        nc.vector.tensor_copy(osb[:], ops[:])
                        nc.sync.dma_start(out.ap()[row0:row0 + 128, nsl], osb[:])

    nc.compile()
    return nc


def _prep_inputs(inputs):
    """Host-side shard + transpose. Returns in_maps for 8 cores."""
    f32 = np.float32
    t = {k: np.asarray(v) for k, v in inputs.items()}
    x_full = {"t": t["trend_emb"].reshape(B * L, D).astype(f32),
              "d": t["detail_emb"].reshape(B * L, D).astype(f32)}
    pT_full = {"t": np.ascontiguousarray(t["trend_proto"].astype(f32).T),
               "d": np.ascontiguousarray(t["detail_proto"].astype(f32).T)}
    W = {("q", "t"): t["t_Wq"], ("q", "d"): t["d_Wq"],
         ("k", "t"): t["t_Wk"], ("k", "d"): t["d_Wk"],
         ("v", "t"): t["t_Wv"], ("v", "d"): t["d_Wv"],
         ("o", "t"): t["t_Wo"], ("o", "d"): t["d_Wo"]}
    bias = {("q", "t"): t["t_bq"], ("q", "d"): t["d_bq"],
            ("k", "t"): t["t_bk"], ("k", "d"): t["d_bk"],
            ("v", "t"): t["t_bv"], ("v", "d"): t["d_bv"],
            ("o", "t"): t["t_bo"], ("o", "d"): t["d_bo"]}

    with_bo = bool(np.any(bias[("o", "t")]) or np.any(bias[("o", "d")]))
    in_maps = []
    for core in range(8):
        rg, hg = divmod(core, HG)
        rows = slice(rg * R, (rg + 1) * R)
        hsl = slice(hg * HEC, (hg + 1) * HEC)
        m = {}
        for b in "td":
            m[f"xT_{b}"] = np.ascontiguousarray(x_full[b][rows].T).astype(bf16)
            m[f"pT_{b}"] = pT_full[b]
            m[f"wq_{b}"] = np.ascontiguousarray(W[("q", b)][:, hsl]).astype(bf16)
            m[f"wk_{b}"] = np.ascontiguousarray(W[("k", b)][:, hsl]).astype(f32)
            m[f"wv_{b}"] = np.ascontiguousarray(W[("v", b)][:, hsl]).astype(f32)
        m["wo"] = np.vstack([W[("o", "t")][hsl, :], W[("o", "d")][hsl, :]]).astype(bf16)
        m["w1"] = t["g_W1"].astype(bf16)
        m["w2"] = t["g_W2"].astype(bf16)
        m["bq2"] = np.stack([bias[("q", "t")][hsl][0:128], bias[("q", "t")][hsl][128:256],
                             bias[("q", "d")][hsl][0:128], bias[("q", "d")][hsl][128:256]],
                            axis=1).astype(f32)
        m["bk2"] = np.stack([bias[("k", "t")][hsl][0:128], bias[("k", "t")][hsl][128:256],
                             bias[("k", "d")][hsl][0:128], bias[("k", "d")][hsl][128:256]],
                            axis=1).astype(f32)
        m["bv"] = np.concatenate([bias[("v", "t")][hsl],
                                  bias[("v", "d")][hsl]])[None, :].astype(f32)
        m["gb1"] = np.ascontiguousarray(
            t["g_b1"].astype(f32).reshape(KD, 128).T)
        m["gb2"] = t["g_b2"].astype(f32).reshape(1, 1)
        m["ones"] = np.ones((1, 2048), f32)
        if with_bo:
            m["bo2"] = (np.stack([bias[("o", "t")], bias[("o", "d")]]) / HG).astype(bf16)
        in_maps.append(m)
    return in_maps, with_bo


def kernel(**inputs):
    global LAST_RESULTS
    import os
    from concourse.bass_utils import run_bass_kernel_spmd

    in_maps, with_bo = _prep_inputs(inputs)
    if with_bo not in _CACHE:
        _CACHE[with_bo] = _build(with_bo)
    nc = _CACHE[with_bo]

    trace = bool(os.environ.get("KERNEL_TRACE"))
    res = run_bass_kernel_spmd(
        nc, in_maps, list(range(8)),
        trace=trace, trace_cores=list(range(8)) if trace else None)
    LAST_RESULTS = res

    out = np.empty((RG, R, DLLM), np.float32)
    for rg in range(RG):
        acc = res.results[rg * HG]["out"].astype(np.float32)
        for hg in range(1, HG):
            acc = acc + res.results[rg * HG + hg]["out"]
        out[rg] = acc
    return out.reshape(B, L, DLLM)


# revision 19
# speedup vs baseline: 1.1917x; 1.1177x over previous
"""DualReprogrammingLayer Trainium2 kernel.

Sharding: 2 row-groups (B*L split in halves) x 4 head-groups (4 heads each).
Each core computes, for its 2048 rows and 4 heads (per block in {trend, detail}):
  KT = (Wk.T @ protoT)           (heads-slice, S)        [K-proj, f32r in, bf16 out]
  V  = (protoT.T @ Wv) + bv      (S, heads-slice)        [V-proj, f32r]
  qT = (Wq.T @ xT) + bq          (heads-slice, rows)     [bf16]
  scoresT = KT_h @ qT_h          (S, rows) per head      [bf16, 2-head row-packed]
  P  = exp(scoresT / 8)                                  [ACT, f32r out]
  A_ext = [V_h | ones].T @ P     (64+64, rows)           [f32r; rows 64:128 = denom]
  gate = sigmoid(relu(cat @ W1) @ W2)  (on-device, bf16)
  A_scaled = A * (gate_coef / denom)                     [bf16]
  out_partial = [A_t; A_d].T-stack @ [Wo_t; Wo_d]        [bf16]
Host sums the 4 head-group partials per row-group.
"""
import sys
sys.path.insert(0, '/opt/trn_rl_repo')
from contextlib import ExitStack

import numpy as np
import ml_dtypes

import concourse.bass as bass
import concourse.tile as tile
from concourse import bacc, mybir

F32 = mybir.dt.float32
F32R = mybir.dt.float32r
BF16 = mybir.dt.bfloat16
AF = mybir.ActivationFunctionType
bf16 = ml_dtypes.bfloat16

B, L, D, S, DLLM, H, E = 4, 1024, 1024, 1000, 4096, 16, 64
RG, HG = 2, 4                 # row-groups x head-groups = 8 cores
R = (B * L) // RG             # 2048 rows per core
NH = H // HG                  # 4 heads per core
HEC = NH * E                  # 256
SCH, NSC = 125, 8             # S = 8 chunks of 125
RC, NRC = 512, 4              # rows = 4 chunks of 512
KD = D // 128                 # 8 k-chunks for d_model
KL = DLLM // 128              # 32 k-chunks for d_llm

_CACHE = {}
LAST_RESULTS = None           # set by kernel(): BassKernelResults


def _build(with_bo):
    nc = bacc.Bacc("TRN2", target_bir_lowering=False, debug=False)

    def din(name, shape, dt):
        return nc.dram_tensor(name, list(shape), dt, kind="ExternalInput")

    xT = {b: din(f"xT_{b}", (D, R), BF16) for b in "td"}
    pT = {b: din(f"pT_{b}", (DLLM, S), BF16) for b in "td"}
    wq = {b: din(f"wq_{b}", (D, HEC), BF16) for b in "td"}
    wk = {b: din(f"wk_{b}", (DLLM, HEC), BF16) for b in "td"}
    wv = {b: din(f"wv_{b}", (DLLM, HEC), BF16) for b in "td"}
    wo = din("wo", (2 * HEC, DLLM), BF16)            # [t rows | d rows]
    w1 = din("w1", (2 * D, D), BF16)
    w2 = din("w2", (D, 1), BF16)
    bq2 = din("bq2", (128, 4), F32)                  # cols: t-mc0, t-mc1, d-mc0, d-mc1
    bk2 = din("bk2", (128, 4), F32)
    bvv = din("bv", (1, 2 * HEC), BF16)               # [t 256 | d 256]
    gb1 = din("gb1", (128, KD), F32)
    gb2 = din("gb2", (1, 1), F32)
    ones_d = din("ones", (1, 2048), F32)
    bo2 = din("bo2", (2, DLLM), BF16) if with_bo else None
    out = nc.dram_tensor("out", [R, DLLM], F32, kind="ExternalOutput")

    with tile.TileContext(nc) as tc, ExitStack() as ctx:
        # ---- persistent pools (live across phases) ----
        pers = ctx.enter_context(tc.tile_pool(name="pers", bufs=1))
        kt_sb = {}    # block -> tile (128, 2, S) bf16 : HE chunk mc at [:, mc, :]
        vx_sb = {}    # block -> tile (125, NSC, NH, 128) f32r : [V_h | ones]
        qt_sb = {}    # block -> tile (128, 2, R) bf16
        for b in "td":
            kt_sb[b] = pers.tile([128, 2, S], BF16, tag=f"kt_{b}", name=f"kt_{b}")
            vx_sb[b] = pers.tile([SCH, NSC, NH, 65], BF16, tag=f"vx_{b}", name=f"vx_{b}")
            qt_sb[b] = pers.tile([128, 2, R], BF16, tag=f"qt_{b}", name=f"qt_{b}")
        gate_sb = pers.tile([1, R], F32, tag="gate")     # sigmoid output
        omg_sb = pers.tile([1, R], F32, tag="omg")       # 1 - gate
        ones125 = pers.tile([1, SCH], BF16, tag="ones125")
        nc.vector.memset(ones125[:], 1.0)
        onesrow = pers.tile([1, RC], F32, tag="onesrow")
        nc.vector.memset(onesrow[:], 1.0)
        bq_sb = pers.tile([128, 4], F32, tag="bq")
        nc.sync.dma_start(bq_sb[:], bq2.ap())
        bk_sb = pers.tile([128, 4], F32, tag="bk")
        nc.sync.dma_start(bk_sb[:], bk2.ap())
        bv_sb = pers.tile([1, 2 * HEC], BF16, tag="bv")
        nc.sync.dma_start(bv_sb[:], bvv.ap())
        gb1_sb = pers.tile([128, KD], F32, tag="gb1")
        nc.sync.dma_start(gb1_sb[:], gb1.ap())
        gb2_sb = pers.tile([1, 1], F32, tag="gb2")
        nc.sync.dma_start(gb2_sb[:], gb2.ap())
        if with_bo:
            bo_sb = pers.tile([2, DLLM], BF16, tag="bo")
            nc.sync.dma_start(bo_sb[:], bo2.ap())
            g2_sb = pers.tile([2, R], BF16, tag="g2")

        # ---- prefetch pools: weights for later phases, loaded during phase A.
        # p_pre2 (wo) lives through phase C; p_pre1 (W1/wq) releases after B.
        p_pre2 = ctx.enter_context(tc.tile_pool(name="p_pre2", bufs=1))
        wo_t = p_pre2.tile([128, 4, DLLM], BF16, tag="wo")
        nc.sync.dma_start(wo_t[:], wo.ap().rearrange("(c p) n -> p c n", p=128))
        pre1ctx = ExitStack()
        p_pre1 = pre1ctx.enter_context(tc.tile_pool(name="p_pre1", bufs=1))
        w1_t = p_pre1.tile([128, 2 * KD, D], BF16, tag="w1")
        nc.sync.dma_start(w1_t[:], w1.ap().rearrange("(c p) m -> p c m", p=128))
        w2_t = p_pre1.tile([128, KD, 1], BF16, tag="w2")
        nc.sync.dma_start(w2_t[:], w2.ap().rearrange("(c p) m -> p c m", p=128))
        wq_t = {}
        for b in "td":
            wq_t[b] = p_pre1.tile([128, KD, HEC], BF16, tag=f"wq_{b}", name=f"wq_{b}")
            nc.sync.dma_start(
                wq_t[b][:], wq[b].ap().rearrange("(c p) m -> p c m", p=128))

        # ---- phase A: K/V projections (proto and weights streamed per k-chunk) ----
        with ExitStack() as actx:
            p_pt = actx.enter_context(tc.tile_pool(name="p_pt", bufs=6))
            p_wc = actx.enter_context(tc.tile_pool(name="p_wc", bufs=1))
            psA = actx.enter_context(tc.tile_pool(name="psA", bufs=1, space="PSUM"))
            for b in "td":
                vps = [psA.tile([SCH, 2, HEC], F32, tag=f"vps{i}", name=f"vps{i}")
                       for i in range(4)]
                kps = [psA.tile([128, 512], F32, tag=f"kps{i}", name=f"kps{i}")
                       for i in range(4)]
                wk_t = p_wc.tile([128, KL, HEC], BF16, tag="wk_t")
                nc.sync.dma_start(
                    wk_t[:], wk[b].ap().rearrange("(c p) m -> p c m", p=128))
                wv_t = p_wc.tile([128, KL, HEC], BF16, tag="wv_t")
                nc.sync.dma_start(
                    wv_t[:], wv[b].ap().rearrange("(c p) m -> p c m", p=128))
                pt_r = pT[b].ap().rearrange("(c p) s -> c p s", c=KL)
                for kc in range(KL):
                    pt_t = p_pt.tile([128, S], BF16, tag="pt")
                    nc.sync.dma_start(pt_t[:], pt_r[kc])
                    wkc = wk_t[:, kc, :]
                    wvc = wv_t[:, kc, :]
                    for si in range(NSC):
                        # one accumulation group per PSUM bank: only the first
                        # half issues start=True (bank-wide clear covers both)
                        nc.tensor.matmul(
                            vps[si // 2][:, si % 2, :],
                            pt_t[:, si * SCH:(si + 1) * SCH],
                            wvc,
                            start=(kc == 0 and si % 2 == 0), stop=False)
                    for mc in range(2):
                        for ncc in range(2):
                            nc.tensor.matmul(
                                kps[mc * 2 + ncc][:, 0:500],
                                wk_t[:, kc, mc * 128:(mc + 1) * 128],
                                pt_t[:, ncc * 500:(ncc + 1) * 500],
                                start=(kc == 0), stop=(kc == KL - 1))
                boff = 0 if b == "t" else HEC
                for si in range(NSC):
                    nc.tensor.matmul(
                        vps[si // 2][:, si % 2, :],
                        ones125[:],
                        bv_sb[:, boff:boff + HEC],
                        start=False, stop=(si % 2 == 1))
                for si in range(NSC):
                    # copy V psum (125, 256) -> [:, si, :, 0:64] viewed as (125, 4, 64)
                    nc.vector.tensor_copy(
                        vx_sb[b][:, si, :, 0:64],
                        vps[si // 2][:, si % 2, :].rearrange("p (h e) -> p h e", h=NH))
                nc.vector.memset(vx_sb[b][:, :, :, 64:65], 1.0)
                for mc in range(2):
                    for ncc in range(2):
                        nc.scalar.activation(
                            kt_sb[b][:, mc, ncc * 500:(ncc + 1) * 500],
                            kps[mc * 2 + ncc][:, 0:500],
                            AF.Identity,
                            bias=bk_sb[:, (0 if b == "t" else 2) + mc:
                                       (0 if b == "t" else 2) + mc + 1])

        # ---- phase B: gate + Q projections (per rows-chunk) ----
        with ExitStack() as bctx:
            p_x = bctx.enter_context(tc.tile_pool(name="p_x", bufs=3))
            p_h = bctx.enter_context(tc.tile_pool(name="p_h", bufs=2))
            psB = bctx.enter_context(tc.tile_pool(name="psB", bufs=2, space="PSUM"))
            for r in range(NRC):
                rsl = slice(r * RC, (r + 1) * RC)
                xt = {}
                for b in "td":
                    xt[b] = p_x.tile([128, KD, RC], BF16, tag=f"x_{b}", name=f"x_{b}")
                    nc.sync.dma_start(
                        xt[b][:],
                        xT[b].ap().rearrange("(c p) n -> p c n", p=128)[:, :, rsl])
                # gate hidden: 8 m-chunks, contraction over 16 chunks (t then d)
                ht = p_h.tile([128, KD, RC], BF16, tag="ht")
                for mc in range(KD):
                    hps = psB.tile([128, RC], F32, tag="hps")
                    for kc in range(2 * KD):
                        nc.tensor.matmul(
                            hps[:],
                            w1_t[:, kc, mc * 128:(mc + 1) * 128],
                            xt["t" if kc < KD else "d"][:, kc % KD, :],
                            start=(kc == 0), stop=(kc == 2 * KD - 1))
                    nc.scalar.activation(
                        ht[:, mc, :], hps[:], AF.Relu,
                        bias=gb1_sb[:, mc:mc + 1])
                lps = psB.tile([1, RC], F32, tag="lps")
                for mc in range(KD):
                    nc.tensor.matmul(
                        lps[:], w2_t[:, mc, :], ht[:, mc, :],
                        start=(mc == 0), stop=(mc == KD - 1))
                nc.scalar.activation(
                    gate_sb[:, rsl], lps[:], AF.Sigmoid, bias=gb2_sb[:])
                nc.vector.tensor_sub(omg_sb[:, rsl], onesrow[:], gate_sb[:, rsl])
                if with_bo:
                    nc.vector.tensor_copy(g2_sb[0:1, rsl], gate_sb[:, rsl])
                    nc.vector.tensor_copy(g2_sb[1:2, rsl], omg_sb[:, rsl])
                # Q projections
                for b in "td":
                    for mc in range(2):
                        qps = psB.tile([128, RC], F32, tag="qps")
                        for kc in range(KD):
                            nc.tensor.matmul(
                                qps[:],
                                wq_t[b][:, kc, mc * 128:(mc + 1) * 128],
                                xt[b][:, kc, :],
                                start=(kc == 0), stop=(kc == KD - 1))
                        nc.scalar.activation(
                            qt_sb[b][:, mc, rsl], qps[:], AF.Identity,
                            bias=bq_sb[:, (0 if b == "t" else 2) + mc:
                                       (0 if b == "t" else 2) + mc + 1])

        pre1ctx.close()

        # ---- phase C: attention + output projection (per rows-chunk) ----
        # Software pipeline over the 4 (block, head-pair) units per rows-chunk:
        # at s-chunk granularity, QK+exp of unit u interleaves with PV of u-1,
        # so the PE never sits idle waiting for ACT's exp.
        with ExitStack() as cctx:
            p_p = cctx.enter_context(tc.tile_pool(name="p_p", bufs=6))
            p_a = cctx.enter_context(tc.tile_pool(name="p_a", bufs=2))
            p_s = cctx.enter_context(tc.tile_pool(name="p_s", bufs=3))
            p_o = cctx.enter_context(tc.tile_pool(name="p_o", bufs=2))
            psS = cctx.enter_context(tc.tile_pool(name="psS", bufs=1, space="PSUM"))
            psPV = cctx.enter_context(tc.tile_pool(name="psPV", bufs=1, space="PSUM"))
            psO = cctx.enter_context(tc.tile_pool(name="psO", bufs=2, space="PSUM"))

            def emit_qk_exp(b, mc, si, rsl):
                sps2 = psS.tile([SCH, 2, RC], F32, tag="sps", name="sps")
                for hh in range(2):  # row-packed pair, adjacent emission
                    po = hh * 64
                    nc.tensor.matmul(
                        sps2[:, hh, :],
                        kt_sb[b][po:po + 64, mc, si * SCH:(si + 1) * SCH],
                        qt_sb[b][po:po + 64, mc, rsl],
                        start=True, stop=True,
                        tile_position=(po, 0))
                p2 = p_p.tile([SCH, 2, RC], BF16, tag=f"p{si % 2}",
                              name=f"p{si % 2}")
                nc.scalar.activation(p2[:], sps2[:], AF.Exp, scale=0.125)
                return p2

            def emit_pv(aps, b, mc, si, p2):
                for hh in range(2):
                    h = mc * 2 + hh
                    nc.tensor.matmul(
                        aps[hh][:], vx_sb[b][:, si, h, :], p2[:, hh, :],
                        start=(si == 0), stop=(si == NSC - 1))

            def emit_norm(aps, b, mc, a2, rsl):
                gcoef = gate_sb if b == "t" else omg_sb
                for hh in range(2):
                    rec1 = p_s.tile([1, RC], F32, tag="rec1")
                    nc.vector.reciprocal(rec1[:], aps[hh][64:65, :])
                    sct1 = p_s.tile([1, RC], F32, tag="sct1")
                    nc.vector.tensor_mul(sct1[:], rec1[:], gcoef[:, rsl])
                    sct64 = p_s.tile([64, RC], F32, tag="sct64")
                    nc.gpsimd.partition_broadcast(sct64[:], sct1[:])
                    nc.vector.tensor_mul(
                        a2[b][mc][hh * 64:hh * 64 + 64, :],
                        aps[hh][0:64, :], sct64[:])

            for r in range(NRC):
                rsl = slice(r * RC, (r + 1) * RC)
                a2 = {b: [p_a.tile([128, RC], BF16, tag=f"a2_{b}{mc}",
                                   name=f"a2_{b}{mc}")
                          for mc in range(2)] for b in "td"}
                units = [(b, mc) for b in "td" for mc in range(2)]
                prev = None   # (aps, b, mc, p2list)
                for b, mc in units:
                    aps = [psPV.tile([65, RC], F32, tag=f"aps{mc}{hh}",
                                     name=f"aps{mc}{hh}") for hh in range(2)]
                    p2buf = {}
                    for si in range(NSC):
                        p2buf[si] = emit_qk_exp(b, mc, si, rsl)
                        if prev is not None:
                            paps, pb, pmc, pp2 = prev
                            emit_pv(paps, pb, pmc, si, pp2[si])
                            pp2[si] = None
                    if prev is not None:
                        emit_norm(prev[0], prev[1], prev[2], a2p, rsl_p)
                    prev = (aps, b, mc, p2buf)
                    a2p, rsl_p = a2, rsl
                # drain last unit of this rows-chunk
                paps, pb, pmc, pp2 = prev
                for si in range(NSC):
                    emit_pv(paps, pb, pmc, si, pp2[si])
                emit_norm(paps, pb, pmc, a2, rsl)

                # output projection for this rows-chunk
                for rb in range(4):
                    row0 = r * RC + rb * 128
                    osb = p_o.tile([128, DLLM], F32, tag="osb")
                    for ncc in range(8):
                        nsl = slice(ncc * 512, (ncc + 1) * 512)
                        ops = psO.tile([128, 512], F32, tag="ops")
                        chains = [("t", 0), ("t", 1), ("d", 0), ("d", 1)]
                        for kk, (bb, mcc) in enumerate(chains):
                            nc.tensor.matmul(
                                ops[:], a2[bb][mcc][:, rb * 128:(rb + 1) * 128],
                                wo_t[:, kk, nsl],
                                start=(kk == 0), stop=(kk == 3 and not with_bo))
                        if with_bo:
                            nc.tensor.matmul(
                                ops[:], g2_sb[:, row0:row0 + 128], bo_sb[:, nsl],
                                start=False, stop=True)
                        nc.vector.tensor_copy(osb[:, nsl], ops[:])
                    nc.sync.dma_start(out.ap()[row0:row0 + 128, :], osb[:])

    nc.compile()
    return nc


def _prep_inputs(inputs):
    """Host-side shard + transpose. Returns in_maps for 8 cores."""
    f32 = np.float32
    t = {k: np.asarray(v) for k, v in inputs.items()}
    x_full = {"t": t["trend_emb"].reshape(B * L, D).astype(f32),
              "d": t["detail_emb"].reshape(B * L, D).astype(f32)}
    pT_full = {"t": np.ascontiguousarray(t["trend_proto"].astype(f32).T).astype(bf16),
               "d": np.ascontiguousarray(t["detail_proto"].astype(f32).T).astype(bf16)}
    W = {("q", "t"): t["t_Wq"], ("q", "d"): t["d_Wq"],
         ("k", "t"): t["t_Wk"], ("k", "d"): t["d_Wk"],
         ("v", "t"): t["t_Wv"], ("v", "d"): t["d_Wv"],
         ("o", "t"): t["t_Wo"], ("o", "d"): t["d_Wo"]}
    bias = {("q", "t"): t["t_bq"], ("q", "d"): t["d_bq"],
            ("k", "t"): t["t_bk"], ("k", "d"): t["d_bk"],
            ("v", "t"): t["t_bv"], ("v", "d"): t["d_bv"],
            ("o", "t"): t["t_bo"], ("o", "d"): t["d_bo"]}

    with_bo = bool(np.any(bias[("o", "t")]) or np.any(bias[("o", "d")]))
    in_maps = []
    for core in range(8):
        rg, hg = divmod(core, HG)
        rows = slice(rg * R, (rg + 1) * R)
        hsl = slice(hg * HEC, (hg + 1) * HEC)
        m = {}
        for b in "td":
            m[f"xT_{b}"] = np.ascontiguousarray(x_full[b][rows].T).astype(bf16)
            m[f"pT_{b}"] = pT_full[b]
            m[f"wq_{b}"] = np.ascontiguousarray(W[("q", b)][:, hsl]).astype(bf16)
            m[f"wk_{b}"] = np.ascontiguousarray(W[("k", b)][:, hsl]).astype(bf16)
            m[f"wv_{b}"] = np.ascontiguousarray(W[("v", b)][:, hsl]).astype(bf16)
        m["wo"] = np.vstack([W[("o", "t")][hsl, :], W[("o", "d")][hsl, :]]).astype(bf16)
        m["w1"] = t["g_W1"].astype(bf16)
        m["w2"] = t["g_W2"].astype(bf16)
        m["bq2"] = np.stack([bias[("q", "t")][hsl][0:128], bias[("q", "t")][hsl][128:256],
                             bias[("q", "d")][hsl][0:128], bias[("q", "d")][hsl][128:256]],
                            axis=1).astype(f32)
        m["bk2"] = np.stack([bias[("k", "t")][hsl][0:128], bias[("k", "t")][hsl][128:256],
                             bias[("k", "d")][hsl][0:128], bias[("k", "d")][hsl][128:256]],
                            axis=1).astype(f32)
        m["bv"] = np.concatenate([bias[("v", "t")][hsl],
                                  bias[("v", "d")][hsl]])[None, :].astype(bf16)
        m["gb1"] = np.ascontiguousarray(
            t["g_b1"].astype(f32).reshape(KD, 128).T)
        m["gb2"] = t["g_b2"].astype(f32).reshape(1, 1)
        m["ones"] = np.ones((1, 2048), f32)
        if with_bo:
            m["bo2"] = (np.stack([bias[("o", "t")], bias[("o", "d")]]) / HG).astype(bf16)
        in_maps.append(m)
    return in_maps, with_bo


def kernel(**inputs):
    global LAST_RESULTS
    import os
    from concourse.bass_utils import run_bass_kernel_spmd

    in_maps, with_bo = _prep_inputs(inputs)
    if with_bo not in _CACHE:
        _CACHE[with_bo] = _build(with_bo)
    nc = _CACHE[with_bo]

    trace = bool(os.environ.get("KERNEL_TRACE"))
    res = run_bass_kernel_spmd(
        nc, in_maps, list(range(8)),
        trace=trace, trace_cores=list(range(8)) if trace else None)
    LAST_RESULTS = res

    out = np.empty((RG, R, DLLM), np.float32)
    for rg in range(RG):
        acc = res.results[rg * HG]["out"].astype(np.float32)
        for hg in range(1, HG):
            acc = acc + res.results[rg * HG + hg]["out"]
        out[rg] = acc
    return out.reshape(B, L, DLLM)


# revision 21
# speedup vs baseline: 1.3821x; 1.1598x over previous
"""DualReprogrammingLayer Trainium2 kernel.

Sharding: 2 row-groups (B*L split in halves) x 4 head-groups (4 heads each).
Each core computes, for its 2048 rows and 4 heads (per block in {trend, detail}):
  KT = (Wk.T @ protoT)           (heads-slice, S)        [K-proj, f32r in, bf16 out]
  V  = (protoT.T @ Wv) + bv      (S, heads-slice)        [V-proj, f32r]
  qT = (Wq.T @ xT) + bq          (heads-slice, rows)     [bf16]
  scoresT = KT_h @ qT_h          (S, rows) per head      [bf16, 2-head row-packed]
  P  = exp(scoresT / 8)                                  [ACT, f32r out]
  A_ext = [V_h | ones].T @ P     (64+64, rows)           [f32r; rows 64:128 = denom]
  gate = sigmoid(relu(cat @ W1) @ W2)  (on-device, bf16)
  A_scaled = A * (gate_coef / denom)                     [bf16]
  out_partial = [A_t; A_d].T-stack @ [Wo_t; Wo_d]        [bf16]
Host sums the 4 head-group partials per row-group.
"""
import sys
sys.path.insert(0, '/opt/trn_rl_repo')
from contextlib import ExitStack

import numpy as np
import ml_dtypes

import concourse.bass as bass
import concourse.tile as tile
from concourse import bacc, mybir

F32 = mybir.dt.float32
F32R = mybir.dt.float32r
BF16 = mybir.dt.bfloat16
AF = mybir.ActivationFunctionType
bf16 = ml_dtypes.bfloat16

B, L, D, S, DLLM, H, E = 4, 1024, 1024, 1000, 4096, 16, 64
RG, HG = 2, 4                 # row-groups x head-groups = 8 cores
R = (B * L) // RG             # 2048 rows per core
NH = H // HG                  # 4 heads per core
HEC = NH * E                  # 256
SCH, NSC = 125, 8             # S = 8 chunks of 125
RC, NRC = 512, 4              # rows = 4 chunks of 512
KD = D // 128                 # 8 k-chunks for d_model
KL = DLLM // 128              # 32 k-chunks for d_llm

_CACHE = {}
LAST_RESULTS = None           # set by kernel(): BassKernelResults


def _build(with_bo):
    nc = bacc.Bacc("TRN2", target_bir_lowering=False, debug=False)

    def din(name, shape, dt):
        return nc.dram_tensor(name, list(shape), dt, kind="ExternalInput")

    xT = {b: din(f"xT_{b}", (D, R), BF16) for b in "td"}
    pT = {b: din(f"pT_{b}", (DLLM, S), BF16) for b in "td"}
    wq = {b: din(f"wq_{b}", (D, HEC), BF16) for b in "td"}
    wk = {b: din(f"wk_{b}", (DLLM, HEC), BF16) for b in "td"}
    wv = {b: din(f"wv_{b}", (DLLM, HEC), BF16) for b in "td"}
    wo = din("wo", (2 * HEC, DLLM), BF16)            # [t rows | d rows]
    w1 = din("w1", (2 * D, D), BF16)
    w2 = din("w2", (D, 1), BF16)
    bq2 = din("bq2", (128, 4), F32)                  # cols: t-mc0, t-mc1, d-mc0, d-mc1
    bk2 = din("bk2", (128, 4), F32)
    bvv = din("bv", (1, 2 * HEC), BF16)               # [t 256 | d 256]
    gb1 = din("gb1", (128, KD), F32)
    gb2 = din("gb2", (1, 1), F32)
    ones_d = din("ones", (1, 2048), F32)
    bo2 = din("bo2", (2, DLLM), BF16) if with_bo else None
    out = nc.dram_tensor("out", [R, DLLM], F32, kind="ExternalOutput")

    with tile.TileContext(nc) as tc, ExitStack() as ctx:
        # ---- persistent pools (live across phases) ----
        pers = ctx.enter_context(tc.tile_pool(name="pers", bufs=1))
        kt_sb = {}    # block -> tile (128, 2, S) bf16 : HE chunk mc at [:, mc, :]
        vx_sb = {}    # block -> tile (125, NSC, NH, 128) f32r : [V_h | ones]
        qt_sb = {}    # block -> tile (128, 2, R) bf16
        for b in "td":
            kt_sb[b] = pers.tile([128, 2, S], BF16, tag=f"kt_{b}", name=f"kt_{b}")
            vx_sb[b] = pers.tile([SCH, NSC, NH, 65], BF16, tag=f"vx_{b}", name=f"vx_{b}")
            qt_sb[b] = pers.tile([128, 2, R], BF16, tag=f"qt_{b}", name=f"qt_{b}")
        gate_sb = pers.tile([1, R], F32, tag="gate")     # sigmoid output
        omg_sb = pers.tile([1, R], F32, tag="omg")       # 1 - gate
        ones125 = pers.tile([1, SCH], BF16, tag="ones125")
        nc.vector.memset(ones125[:], 1.0)
        onesrow = pers.tile([1, RC], F32, tag="onesrow")
        nc.vector.memset(onesrow[:], 1.0)
        bq_sb = pers.tile([128, 4], F32, tag="bq")
        nc.sync.dma_start(bq_sb[:], bq2.ap())
        bk_sb = pers.tile([128, 4], F32, tag="bk")
        nc.sync.dma_start(bk_sb[:], bk2.ap())
        bv_sb = pers.tile([1, 2 * HEC], BF16, tag="bv")
        nc.sync.dma_start(bv_sb[:], bvv.ap())
        gb1_sb = pers.tile([128, KD], F32, tag="gb1")
        nc.sync.dma_start(gb1_sb[:], gb1.ap())
        gb2_sb = pers.tile([1, 1], F32, tag="gb2")
        nc.sync.dma_start(gb2_sb[:], gb2.ap())
        if with_bo:
            bo_sb = pers.tile([2, DLLM], BF16, tag="bo")
            nc.sync.dma_start(bo_sb[:], bo2.ap())
            g2_sb = pers.tile([2, R], BF16, tag="g2")

        # ---- prefetch pools: weights for later phases, loaded during phase A.
        # p_pre2 (wo) lives through phase C; p_pre1 (W1/wq) releases after B.
        p_pre2 = ctx.enter_context(tc.tile_pool(name="p_pre2", bufs=1))
        wo_t = p_pre2.tile([128, 4, DLLM], BF16, tag="wo")
        nc.sync.dma_start(wo_t[:], wo.ap().rearrange("(c p) n -> p c n", p=128))
        pre1ctx = ExitStack()
        p_pre1 = pre1ctx.enter_context(tc.tile_pool(name="p_pre1", bufs=1))
        w1_t = p_pre1.tile([128, 2 * KD, D], BF16, tag="w1")
        nc.sync.dma_start(w1_t[:], w1.ap().rearrange("(c p) m -> p c m", p=128))
        w2_t = p_pre1.tile([128, KD, 1], BF16, tag="w2")
        nc.sync.dma_start(w2_t[:], w2.ap().rearrange("(c p) m -> p c m", p=128))
        wq_t = {}
        for b in "td":
            wq_t[b] = p_pre1.tile([128, KD, HEC], BF16, tag=f"wq_{b}", name=f"wq_{b}")
            nc.sync.dma_start(
                wq_t[b][:], wq[b].ap().rearrange("(c p) m -> p c m", p=128))

        # ---- phase A: K/V projections (proto and weights streamed per k-chunk) ----
        with ExitStack() as actx:
            p_pt = actx.enter_context(tc.tile_pool(name="p_pt", bufs=6))
            p_wc = actx.enter_context(tc.tile_pool(name="p_wc", bufs=1))
            psA = actx.enter_context(tc.tile_pool(name="psA", bufs=1, space="PSUM"))
            for b in "td":
                vps = [psA.tile([SCH, 2, HEC], F32, tag=f"vps{i}", name=f"vps{i}")
                       for i in range(4)]
                kps = [psA.tile([128, 512], F32, tag=f"kps{i}", name=f"kps{i}")
                       for i in range(4)]
                wk_t = p_wc.tile([128, KL, HEC], BF16, tag="wk_t")
                nc.sync.dma_start(
                    wk_t[:], wk[b].ap().rearrange("(c p) m -> p c m", p=128))
                wv_t = p_wc.tile([128, KL, HEC], BF16, tag="wv_t")
                nc.sync.dma_start(
                    wv_t[:], wv[b].ap().rearrange("(c p) m -> p c m", p=128))
                pt_r = pT[b].ap().rearrange("(c p) s -> c p s", c=KL)
                for kc in range(KL):
                    pt_t = p_pt.tile([128, S], BF16, tag="pt")
                    nc.sync.dma_start(pt_t[:], pt_r[kc])
                    wkc = wk_t[:, kc, :]
                    wvc = wv_t[:, kc, :]
                    for si in range(NSC):
                        # one accumulation group per PSUM bank: only the first
                        # half issues start=True (bank-wide clear covers both)
                        nc.tensor.matmul(
                            vps[si // 2][:, si % 2, :],
                            pt_t[:, si * SCH:(si + 1) * SCH],
                            wvc,
                            start=(kc == 0 and si % 2 == 0), stop=False)
                    for mc in range(2):
                        for ncc in range(2):
                            nc.tensor.matmul(
                                kps[mc * 2 + ncc][:, 0:500],
                                wk_t[:, kc, mc * 128:(mc + 1) * 128],
                                pt_t[:, ncc * 500:(ncc + 1) * 500],
                                start=(kc == 0), stop=(kc == KL - 1))
                boff = 0 if b == "t" else HEC
                for si in range(NSC):
                    nc.tensor.matmul(
                        vps[si // 2][:, si % 2, :],
                        ones125[:],
                        bv_sb[:, boff:boff + HEC],
                        start=False, stop=(si % 2 == 1))
                for si in range(NSC):
                    # copy V psum (125, 256) -> [:, si, :, 0:64] viewed as (125, 4, 64)
                    nc.vector.tensor_copy(
                        vx_sb[b][:, si, :, 0:64],
                        vps[si // 2][:, si % 2, :].rearrange("p (h e) -> p h e", h=NH))
                nc.vector.memset(vx_sb[b][:, :, :, 64:65], 1.0)
                for mc in range(2):
                    for ncc in range(2):
                        nc.scalar.activation(
                            kt_sb[b][:, mc, ncc * 500:(ncc + 1) * 500],
                            kps[mc * 2 + ncc][:, 0:500],
                            AF.Identity,
                            bias=bk_sb[:, (0 if b == "t" else 2) + mc:
                                       (0 if b == "t" else 2) + mc + 1])

        # ---- phase B: gate + Q projections (per rows-chunk) ----
        with ExitStack() as bctx:
            p_x = bctx.enter_context(tc.tile_pool(name="p_x", bufs=3))
            p_h = bctx.enter_context(tc.tile_pool(name="p_h", bufs=2))
            psB = bctx.enter_context(tc.tile_pool(name="psB", bufs=2, space="PSUM"))
            for r in range(NRC):
                rsl = slice(r * RC, (r + 1) * RC)
                xt = {}
                for b in "td":
                    xt[b] = p_x.tile([128, KD, RC], BF16, tag=f"x_{b}", name=f"x_{b}")
                    nc.sync.dma_start(
                        xt[b][:],
                        xT[b].ap().rearrange("(c p) n -> p c n", p=128)[:, :, rsl])
                # gate hidden: 8 m-chunks, contraction over 16 chunks (t then d)
                ht = p_h.tile([128, KD, RC], BF16, tag="ht")
                for mc in range(KD):
                    hps = psB.tile([128, RC], F32, tag="hps")
                    for kc in range(2 * KD):
                        nc.tensor.matmul(
                            hps[:],
                            w1_t[:, kc, mc * 128:(mc + 1) * 128],
                            xt["t" if kc < KD else "d"][:, kc % KD, :],
                            start=(kc == 0), stop=(kc == 2 * KD - 1))
                    nc.scalar.activation(
                        ht[:, mc, :], hps[:], AF.Relu,
                        bias=gb1_sb[:, mc:mc + 1])
                lps = psB.tile([1, RC], F32, tag="lps")
                for mc in range(KD):
                    nc.tensor.matmul(
                        lps[:], w2_t[:, mc, :], ht[:, mc, :],
                        start=(mc == 0), stop=(mc == KD - 1))
                nc.scalar.activation(
                    gate_sb[:, rsl], lps[:], AF.Sigmoid, bias=gb2_sb[:])
                nc.vector.tensor_sub(omg_sb[:, rsl], onesrow[:], gate_sb[:, rsl])
                if with_bo:
                    nc.vector.tensor_copy(g2_sb[0:1, rsl], gate_sb[:, rsl])
                    nc.vector.tensor_copy(g2_sb[1:2, rsl], omg_sb[:, rsl])
                # Q projections
                for b in "td":
                    for mc in range(2):
                        qps = psB.tile([128, RC], F32, tag="qps")
                        for kc in range(KD):
                            nc.tensor.matmul(
                                qps[:],
                                wq_t[b][:, kc, mc * 128:(mc + 1) * 128],
                                xt[b][:, kc, :],
                                start=(kc == 0), stop=(kc == KD - 1))
                        nc.scalar.activation(
                            qt_sb[b][:, mc, rsl], qps[:], AF.Identity,
                            bias=bq_sb[:, (0 if b == "t" else 2) + mc:
                                       (0 if b == "t" else 2) + mc + 1])

        pre1ctx.close()

        # ---- phase C: attention + output projection (per rows-chunk) ----
        # Software pipeline over the 4 (block, head-pair) units per rows-chunk:
        # at s-chunk granularity, QK+exp of unit u interleaves with PV of u-1,
        # so the PE never sits idle waiting for ACT's exp.
        with ExitStack() as cctx:
            p_p = cctx.enter_context(tc.tile_pool(name="p_p", bufs=6))
            p_a = cctx.enter_context(tc.tile_pool(name="p_a", bufs=2))
            p_s = cctx.enter_context(tc.tile_pool(name="p_s", bufs=3))
            p_o = cctx.enter_context(tc.tile_pool(name="p_o", bufs=2))
            psS = cctx.enter_context(tc.tile_pool(name="psS", bufs=1, space="PSUM"))
            psPV = cctx.enter_context(tc.tile_pool(name="psPV", bufs=1, space="PSUM"))
            psO = cctx.enter_context(tc.tile_pool(name="psO", bufs=2, space="PSUM"))

            def emit_qk_exp(b, mc, si, rsl):
                sps2 = psS.tile([SCH, 2, RC], F32, tag="sps", name="sps")
                for hh in range(2):  # row-packed pair, adjacent emission
                    po = hh * 64
                    nc.tensor.matmul(
                        sps2[:, hh, :],
                        kt_sb[b][po:po + 64, mc, si * SCH:(si + 1) * SCH],
                        qt_sb[b][po:po + 64, mc, rsl],
                        start=True, stop=True,
                        tile_position=(po, 0))
                p2 = p_p.tile([SCH, 2, RC], BF16, tag=f"p{si % 2}",
                              name=f"p{si % 2}")
                nc.scalar.activation(p2[:], sps2[:], AF.Exp, scale=0.125)
                return p2

            def emit_pv(aps, b, mc, si, p2):
                for hh in range(2):
                    h = mc * 2 + hh
                    nc.tensor.matmul(
                        aps[hh][:], vx_sb[b][:, si, h, :], p2[:, hh, :],
                        start=(si == 0), stop=(si == NSC - 1))

            def emit_norm(aps, b, mc, a2, rsl):
                gcoef = gate_sb if b == "t" else omg_sb
                for hh in range(2):
                    den1 = p_s.tile([1, RC], F32, tag="den1")
                    nc.vector.tensor_copy(den1[:], aps[hh][64:65, :])
                    rec1 = p_s.tile([1, RC], F32, tag="rec1")
                    nc.vector.reciprocal_approx_fast(rec1[:], den1[:])
                    sct1 = p_s.tile([1, RC], F32, tag="sct1")
                    nc.vector.tensor_mul(sct1[:], rec1[:], gcoef[:, rsl])
                    sct64 = p_s.tile([64, RC], F32, tag="sct64")
                    nc.gpsimd.partition_broadcast(sct64[:], sct1[:])
                    nc.vector.tensor_mul(
                        a2[b][mc][hh * 64:hh * 64 + 64, :],
                        aps[hh][0:64, :], sct64[:])

            for r in range(NRC):
                rsl = slice(r * RC, (r + 1) * RC)
                a2 = {b: [p_a.tile([128, RC], BF16, tag=f"a2_{b}{mc}",
                                   name=f"a2_{b}{mc}")
                          for mc in range(2)] for b in "td"}
                units = [(b, mc) for b in "td" for mc in range(2)]
                prev = None   # (aps, b, mc, p2list)
                for b, mc in units:
                    aps = [psPV.tile([65, RC], F32, tag=f"aps{mc}{hh}",
                                     name=f"aps{mc}{hh}") for hh in range(2)]
                    p2buf = {}
                    for si in range(NSC):
                        p2buf[si] = emit_qk_exp(b, mc, si, rsl)
                        if prev is not None:
                            paps, pb, pmc, pp2 = prev
                            emit_pv(paps, pb, pmc, si, pp2[si])
                            pp2[si] = None
                    if prev is not None:
                        emit_norm(prev[0], prev[1], prev[2], a2p, rsl_p)
                    prev = (aps, b, mc, p2buf)
                    a2p, rsl_p = a2, rsl
                # drain last unit of this rows-chunk
                paps, pb, pmc, pp2 = prev
                for si in range(NSC):
                    emit_pv(paps, pb, pmc, si, pp2[si])
                emit_norm(paps, pb, pmc, a2, rsl)

                # output projection for this rows-chunk
                for rb in range(4):
                    row0 = r * RC + rb * 128
                    osb = p_o.tile([128, DLLM], F32, tag="osb")
                    for ncc in range(8):
                        nsl = slice(ncc * 512, (ncc + 1) * 512)
                        ops = psO.tile([128, 512], F32, tag="ops")
                        chains = [("t", 0), ("t", 1), ("d", 0), ("d", 1)]
                        for kk, (bb, mcc) in enumerate(chains):
                            nc.tensor.matmul(
                                ops[:], a2[bb][mcc][:, rb * 128:(rb + 1) * 128],
                                wo_t[:, kk, nsl],
                                start=(kk == 0), stop=(kk == 3 and not with_bo))
                        if with_bo:
                            nc.tensor.matmul(
                                ops[:], g2_sb[:, row0:row0 + 128], bo_sb[:, nsl],
                                start=False, stop=True)
                        nc.vector.tensor_copy(osb[:, nsl], ops[:])
                    nc.sync.dma_start(out.ap()[row0:row0 + 128, :], osb[:])

    nc.compile()
    return nc


def _prep_inputs(inputs):
    """Host-side shard + transpose. Returns in_maps for 8 cores."""
    f32 = np.float32
    t = {k: np.asarray(v) for k, v in inputs.items()}
    x_full = {"t": t["trend_emb"].reshape(B * L, D).astype(f32),
              "d": t["detail_emb"].reshape(B * L, D).astype(f32)}
    pT_full = {"t": np.ascontiguousarray(t["trend_proto"].astype(f32).T).astype(bf16),
               "d": np.ascontiguousarray(t["detail_proto"].astype(f32).T).astype(bf16)}
    W = {("q", "t"): t["t_Wq"], ("q", "d"): t["d_Wq"],
         ("k", "t"): t["t_Wk"], ("k", "d"): t["d_Wk"],
         ("v", "t"): t["t_Wv"], ("v", "d"): t["d_Wv"],
         ("o", "t"): t["t_Wo"], ("o", "d"): t["d_Wo"]}
    bias = {("q", "t"): t["t_bq"], ("q", "d"): t["d_bq"],
            ("k", "t"): t["t_bk"], ("k", "d"): t["d_bk"],
            ("v", "t"): t["t_bv"], ("v", "d"): t["d_bv"],
            ("o", "t"): t["t_bo"], ("o", "d"): t["d_bo"]}

    with_bo = bool(np.any(bias[("o", "t")]) or np.any(bias[("o", "d")]))
    in_maps = []
    for core in range(8):
        rg, hg = divmod(core, HG)
        rows = slice(rg * R, (rg + 1) * R)
        hsl = slice(hg * HEC, (hg + 1) * HEC)
        m = {}
        for b in "td":
            m[f"xT_{b}"] = np.ascontiguousarray(x_full[b][rows].T).astype(bf16)
            m[f"pT_{b}"] = pT_full[b]
            m[f"wq_{b}"] = np.ascontiguousarray(W[("q", b)][:, hsl]).astype(bf16)
            m[f"wk_{b}"] = np.ascontiguousarray(W[("k", b)][:, hsl]).astype(bf16)
            m[f"wv_{b}"] = np.ascontiguousarray(W[("v", b)][:, hsl]).astype(bf16)
        m["wo"] = np.vstack([W[("o", "t")][hsl, :], W[("o", "d")][hsl, :]]).astype(bf16)
        m["w1"] = t["g_W1"].astype(bf16)
        m["w2"] = t["g_W2"].astype(bf16)
        m["bq2"] = np.stack([bias[("q", "t")][hsl][0:128], bias[("q", "t")][hsl][128:256],
                             bias[("q", "d")][hsl][0:128], bias[("q", "d")][hsl][128:256]],
                            axis=1).astype(f32)
        m["bk2"] = np.stack([bias[("k", "t")][hsl][0:128], bias[("k", "t")][hsl][128:256],
                             bias[("k", "d")][hsl][0:128], bias[("k", "d")][hsl][128:256]],
                            axis=1).astype(f32)
        m["bv"] = np.concatenate([bias[("v", "t")][hsl],
                                  bias[("v", "d")][hsl]])[None, :].astype(bf16)
        m["gb1"] = np.ascontiguousarray(
            t["g_b1"].astype(f32).reshape(KD, 128).T)
        m["gb2"] = t["g_b2"].astype(f32).reshape(1, 1)
        m["ones"] = np.ones((1, 2048), f32)
        if with_bo:
            m["bo2"] = (np.stack([bias[("o", "t")], bias[("o", "d")]]) / HG).astype(bf16)
        in_maps.append(m)
    return in_maps, with_bo


def kernel(**inputs):
    global LAST_RESULTS
    import os
    from concourse.bass_utils import run_bass_kernel_spmd

    in_maps, with_bo = _prep_inputs(inputs)
    if with_bo not in _CACHE:
        _CACHE[with_bo] = _build(with_bo)
    nc = _CACHE[with_bo]

    trace = bool(os.environ.get("KERNEL_TRACE"))
    res = run_bass_kernel_spmd(
        nc, in_maps, list(range(8)),
        trace=trace, trace_cores=list(range(8)) if trace else None)
    LAST_RESULTS = res

    out = np.empty((RG, R, DLLM), np.float32)
    for rg in range(RG):
        acc = res.results[rg * HG]["out"].astype(np.float32)
        for hg in range(1, HG):
            acc = acc + res.results[rg * HG + hg]["out"]
        out[rg] = acc
    return out.reshape(B, L, DLLM)


# revision 23
# speedup vs baseline: 1.5550x; 1.1251x over previous
"""DualReprogrammingLayer Trainium2 kernel.

Sharding: 2 row-groups (B*L split in halves) x 4 head-groups (4 heads each).
Each core computes, for its 2048 rows and 4 heads (per block in {trend, detail}):
  KT = (Wk.T @ protoT)           (heads-slice, S)        [K-proj, f32r in, bf16 out]
  V  = (protoT.T @ Wv) + bv      (S, heads-slice)        [V-proj, f32r]
  qT = (Wq.T @ xT) + bq          (heads-slice, rows)     [bf16]
  scoresT = KT_h @ qT_h          (S, rows) per head      [bf16, 2-head row-packed]
  P  = exp(scoresT / 8)                                  [ACT, f32r out]
  A_ext = [V_h | ones].T @ P     (64+64, rows)           [f32r; rows 64:128 = denom]
  gate = sigmoid(relu(cat @ W1) @ W2)  (on-device, bf16)
  A_scaled = A * (gate_coef / denom)                     [bf16]
  out_partial = [A_t; A_d].T-stack @ [Wo_t; Wo_d]        [bf16]
Host sums the 4 head-group partials per row-group.
"""
import sys
sys.path.insert(0, '/opt/trn_rl_repo')
from contextlib import ExitStack

import numpy as np
import ml_dtypes

import concourse.bass as bass
import concourse.tile as tile
from concourse import bacc, mybir

F32 = mybir.dt.float32
F32R = mybir.dt.float32r
BF16 = mybir.dt.bfloat16
AF = mybir.ActivationFunctionType
bf16 = ml_dtypes.bfloat16

B, L, D, S, DLLM, H, E = 4, 1024, 1024, 1000, 4096, 16, 64
RG, HG = 2, 4                 # row-groups x head-groups = 8 cores
R = (B * L) // RG             # 2048 rows per core
NH = H // HG                  # 4 heads per core
HEC = NH * E                  # 256
SCH, NSC = 125, 8             # S = 8 chunks of 125
RC, NRC = 512, 4              # rows = 4 chunks of 512
KD = D // 128                 # 8 k-chunks for d_model
KL = DLLM // 128              # 32 k-chunks for d_llm

_CACHE = {}
LAST_RESULTS = None           # set by kernel(): BassKernelResults


def _build(with_bo):
    nc = bacc.Bacc("TRN2", target_bir_lowering=False, debug=False)

    def din(name, shape, dt):
        return nc.dram_tensor(name, list(shape), dt, kind="ExternalInput")

    xT = {b: din(f"xT_{b}", (D, R), BF16) for b in "td"}
    pT = {b: din(f"pT_{b}", (DLLM, S), BF16) for b in "td"}
    wq = {b: din(f"wq_{b}", (D, HEC), BF16) for b in "td"}
    wk = {b: din(f"wk_{b}", (DLLM, HEC), BF16) for b in "td"}
    wv = {b: din(f"wv_{b}", (DLLM, HEC), BF16) for b in "td"}
    wo = din("wo", (2 * HEC, DLLM), BF16)            # [t rows | d rows]
    w1 = din("w1", (2 * D, D), BF16)
    w2 = din("w2", (D, 1), BF16)
    bq2 = din("bq2", (128, 4), F32)                  # cols: t-mc0, t-mc1, d-mc0, d-mc1
    bk2 = din("bk2", (128, 4), F32)
    bvv = din("bv", (1, 2 * HEC), BF16)               # [t 256 | d 256]
    gb1 = din("gb1", (128, KD), F32)
    gb2 = din("gb2", (1, 1), F32)
    ones_d = din("ones", (1, 2048), F32)
    bo2 = din("bo2", (2, DLLM), BF16) if with_bo else None
    out = nc.dram_tensor("out", [R, DLLM], F32, kind="ExternalOutput")

    with tile.TileContext(nc) as tc, ExitStack() as ctx:
        # ---- persistent pools (live across phases) ----
        pers = ctx.enter_context(tc.tile_pool(name="pers", bufs=1))
        kt_sb = {}    # block -> tile (128, 2, S) bf16 : HE chunk mc at [:, mc, :]
        vx_sb = {}    # block -> tile (125, NSC, NH, 128) f32r : [V_h | ones]
        qt_sb = {}    # block -> tile (128, 2, R) bf16
        for b in "td":
            kt_sb[b] = pers.tile([128, 2, S], BF16, tag=f"kt_{b}", name=f"kt_{b}")
            vx_sb[b] = pers.tile([SCH, NSC, NH, 65], BF16, tag=f"vx_{b}", name=f"vx_{b}")
            qt_sb[b] = pers.tile([128, 2, R], BF16, tag=f"qt_{b}", name=f"qt_{b}")
        gate_sb = pers.tile([1, R], F32, tag="gate")     # sigmoid output
        omg_sb = pers.tile([1, R], F32, tag="omg")       # 1 - gate
        ones125 = pers.tile([1, SCH], BF16, tag="ones125")
        nc.vector.memset(ones125[:], 1.0)
        onesrow = pers.tile([1, RC], F32, tag="onesrow")
        nc.vector.memset(onesrow[:], 1.0)
        bq_sb = pers.tile([128, 4], F32, tag="bq")
        nc.sync.dma_start(bq_sb[:], bq2.ap())
        bk_sb = pers.tile([128, 4], F32, tag="bk")
        nc.sync.dma_start(bk_sb[:], bk2.ap())
        bv_sb = pers.tile([1, 2 * HEC], BF16, tag="bv")
        nc.sync.dma_start(bv_sb[:], bvv.ap())
        gb1_sb = pers.tile([128, KD], F32, tag="gb1")
        nc.sync.dma_start(gb1_sb[:], gb1.ap())
        gb2_sb = pers.tile([1, 1], F32, tag="gb2")
        nc.sync.dma_start(gb2_sb[:], gb2.ap())
        if with_bo:
            bo_sb = pers.tile([2, DLLM], BF16, tag="bo")
            nc.sync.dma_start(bo_sb[:], bo2.ap())
            g2_sb = pers.tile([2, R], BF16, tag="g2")

        # ---- prefetch pools: weights for later phases, loaded during phase A.
        # p_pre2 (wo) lives through phase C; p_pre1 (W1/wq) releases after B.
        p_pre2 = ctx.enter_context(tc.tile_pool(name="p_pre2", bufs=1))
        wo_t = p_pre2.tile([128, 4, DLLM], BF16, tag="wo")
        nc.sync.dma_start(wo_t[:], wo.ap().rearrange("(c p) n -> p c n", p=128))
        pre1ctx = ExitStack()
        p_pre1 = pre1ctx.enter_context(tc.tile_pool(name="p_pre1", bufs=1))
        w1_t = p_pre1.tile([128, 2 * KD, D], BF16, tag="w1")
        nc.sync.dma_start(w1_t[:], w1.ap().rearrange("(c p) m -> p c m", p=128))
        w2_t = p_pre1.tile([128, KD, 1], BF16, tag="w2")
        nc.sync.dma_start(w2_t[:], w2.ap().rearrange("(c p) m -> p c m", p=128))
        wq_t = {}
        for b in "td":
            wq_t[b] = p_pre1.tile([128, KD, HEC], BF16, tag=f"wq_{b}", name=f"wq_{b}")
            nc.sync.dma_start(
                wq_t[b][:], wq[b].ap().rearrange("(c p) m -> p c m", p=128))

        # ---- phase A: K/V projections (proto and weights streamed per k-chunk) ----
        with ExitStack() as actx:
            p_pt = actx.enter_context(tc.tile_pool(name="p_pt", bufs=6))
            p_wc = actx.enter_context(tc.tile_pool(name="p_wc", bufs=1))
            psA = actx.enter_context(tc.tile_pool(name="psA", bufs=1, space="PSUM"))
            for b in "td":
                vps = [psA.tile([SCH, 2, HEC], F32, tag=f"vps{i}", name=f"vps{i}")
                       for i in range(4)]
                kps = [psA.tile([128, 512], F32, tag=f"kps{i}", name=f"kps{i}")
                       for i in range(4)]
                wk_t = p_wc.tile([128, KL, HEC], BF16, tag="wk_t")
                nc.sync.dma_start(
                    wk_t[:], wk[b].ap().rearrange("(c p) m -> p c m", p=128))
                wv_t = p_wc.tile([128, KL, HEC], BF16, tag="wv_t")
                nc.sync.dma_start(
                    wv_t[:], wv[b].ap().rearrange("(c p) m -> p c m", p=128))
                pt_r = pT[b].ap().rearrange("(c p) s -> c p s", c=KL)
                for kc in range(KL):
                    pt_t = p_pt.tile([128, S], BF16, tag="pt")
                    nc.sync.dma_start(pt_t[:], pt_r[kc])
                    wkc = wk_t[:, kc, :]
                    wvc = wv_t[:, kc, :]
                    for si in range(NSC):
                        # one accumulation group per PSUM bank: only the first
                        # half issues start=True (bank-wide clear covers both)
                        nc.tensor.matmul(
                            vps[si // 2][:, si % 2, :],
                            pt_t[:, si * SCH:(si + 1) * SCH],
                            wvc,
                            start=(kc == 0 and si % 2 == 0), stop=False)
                    for mc in range(2):
                        for ncc in range(2):
                            nc.tensor.matmul(
                                kps[mc * 2 + ncc][:, 0:500],
                                wk_t[:, kc, mc * 128:(mc + 1) * 128],
                                pt_t[:, ncc * 500:(ncc + 1) * 500],
                                start=(kc == 0), stop=(kc == KL - 1))
                boff = 0 if b == "t" else HEC
                for si in range(NSC):
                    nc.tensor.matmul(
                        vps[si // 2][:, si % 2, :],
                        ones125[:],
                        bv_sb[:, boff:boff + HEC],
                        start=False, stop=(si % 2 == 1))
                for si in range(NSC):
                    # copy V psum (125, 256) -> [:, si, :, 0:64] viewed as (125, 4, 64)
                    nc.vector.tensor_copy(
                        vx_sb[b][:, si, :, 0:64],
                        vps[si // 2][:, si % 2, :].rearrange("p (h e) -> p h e", h=NH))
                nc.vector.memset(vx_sb[b][:, :, :, 64:65], 1.0)
                for mc in range(2):
                    for ncc in range(2):
                        nc.scalar.activation(
                            kt_sb[b][:, mc, ncc * 500:(ncc + 1) * 500],
                            kps[mc * 2 + ncc][:, 0:500],
                            AF.Identity,
                            bias=bk_sb[:, (0 if b == "t" else 2) + mc:
                                       (0 if b == "t" else 2) + mc + 1])

        # ---- phase B: gate + Q projections (per rows-chunk) ----
        with ExitStack() as bctx:
            p_x = bctx.enter_context(tc.tile_pool(name="p_x", bufs=3))
            p_h = bctx.enter_context(tc.tile_pool(name="p_h", bufs=2))
            psB = bctx.enter_context(tc.tile_pool(name="psB", bufs=2, space="PSUM"))
            for r in range(NRC):
                rsl = slice(r * RC, (r + 1) * RC)
                xt = {}
                for b in "td":
                    xt[b] = p_x.tile([128, KD, RC], BF16, tag=f"x_{b}", name=f"x_{b}")
                    nc.sync.dma_start(
                        xt[b][:],
                        xT[b].ap().rearrange("(c p) n -> p c n", p=128)[:, :, rsl])
                # gate hidden: 8 m-chunks, contraction over 16 chunks (t then d)
                ht = p_h.tile([128, KD, RC], BF16, tag="ht")
                for mc in range(KD):
                    hps = psB.tile([128, RC], F32, tag="hps")
                    for kc in range(2 * KD):
                        nc.tensor.matmul(
                            hps[:],
                            w1_t[:, kc, mc * 128:(mc + 1) * 128],
                            xt["t" if kc < KD else "d"][:, kc % KD, :],
                            start=(kc == 0), stop=(kc == 2 * KD - 1))
                    nc.scalar.activation(
                        ht[:, mc, :], hps[:], AF.Relu,
                        bias=gb1_sb[:, mc:mc + 1])
                lps = psB.tile([1, RC], F32, tag="lps")
                for mc in range(KD):
                    nc.tensor.matmul(
                        lps[:], w2_t[:, mc, :], ht[:, mc, :],
                        start=(mc == 0), stop=(mc == KD - 1))
                nc.scalar.activation(
                    gate_sb[:, rsl], lps[:], AF.Sigmoid, bias=gb2_sb[:])
                nc.vector.tensor_sub(omg_sb[:, rsl], onesrow[:], gate_sb[:, rsl])
                if with_bo:
                    nc.vector.tensor_copy(g2_sb[0:1, rsl], gate_sb[:, rsl])
                    nc.vector.tensor_copy(g2_sb[1:2, rsl], omg_sb[:, rsl])
                # Q projections
                for b in "td":
                    for mc in range(2):
                        qps = psB.tile([128, RC], F32, tag="qps")
                        for kc in range(KD):
                            nc.tensor.matmul(
                                qps[:],
                                wq_t[b][:, kc, mc * 128:(mc + 1) * 128],
                                xt[b][:, kc, :],
                                start=(kc == 0), stop=(kc == KD - 1))
                        nc.scalar.activation(
                            qt_sb[b][:, mc, rsl], qps[:], AF.Identity,
                            bias=bq_sb[:, (0 if b == "t" else 2) + mc:
                                       (0 if b == "t" else 2) + mc + 1])
        pre1ctx.close()

        # ---- phase C: attention + output projection ----
        # Software pipeline: QK+exp of unit u overlaps PV of unit u-1 at
        # s-chunk granularity; output-projection rows-blocks of the previous
        # rows-chunk interleave between units to keep the PE stream dense.
        with ExitStack() as cctx:
            p_p = cctx.enter_context(tc.tile_pool(name="p_p", bufs=6))
            p_a = cctx.enter_context(tc.tile_pool(name="p_a", bufs=2))
            p_s = cctx.enter_context(tc.tile_pool(name="p_s", bufs=3))
            p_o = cctx.enter_context(tc.tile_pool(name="p_o", bufs=2))
            psS = cctx.enter_context(tc.tile_pool(name="psS", bufs=1, space="PSUM"))
            psPV = cctx.enter_context(tc.tile_pool(name="psPV", bufs=1, space="PSUM"))
            psO = cctx.enter_context(tc.tile_pool(name="psO", bufs=2, space="PSUM"))

            def emit_qk_exp(b, mc, si, rsl):
                sps2 = psS.tile([SCH, 2, RC], F32, tag="sps", name="sps")
                for hh in range(2):  # row-packed pair, adjacent emission
                    po = hh * 64
                    nc.tensor.matmul(
                        sps2[:, hh, :],
                        kt_sb[b][po:po + 64, mc, si * SCH:(si + 1) * SCH],
                        qt_sb[b][po:po + 64, mc, rsl],
                        start=True, stop=True,
                        tile_position=(po, 0))
                p2 = p_p.tile([SCH, 2, RC], BF16, tag=f"p{si % 2}",
                              name=f"p{si % 2}")
                nc.scalar.activation(p2[:], sps2[:], AF.Exp, scale=0.125)
                return p2

            def emit_pv(aps, b, mc, si, p2):
                for hh in range(2):
                    h = mc * 2 + hh
                    nc.tensor.matmul(
                        aps[hh][:], vx_sb[b][:, si, h, :], p2[:, hh, :],
                        start=(si == 0), stop=(si == NSC - 1))

            def emit_norm(aps, b, mc, a2, rsl):
                gcoef = gate_sb if b == "t" else omg_sb
                for hh in range(2):
                    den1 = p_s.tile([1, RC], F32, tag="den1")
                    nc.vector.tensor_copy(den1[:], aps[hh][64:65, :])
                    rec1 = p_s.tile([1, RC], F32, tag="rec1")
                    nc.vector.reciprocal_approx_fast(rec1[:], den1[:])
                    sct1 = p_s.tile([1, RC], F32, tag="sct1")
                    nc.vector.tensor_mul(sct1[:], rec1[:], gcoef[:, rsl])
                    sct64 = p_s.tile([64, RC], F32, tag="sct64")
                    nc.gpsimd.partition_broadcast(sct64[:], sct1[:])
                    nc.vector.tensor_mul(
                        a2[b][mc][hh * 64:hh * 64 + 64, :],
                        aps[hh][0:64, :], sct64[:])

            def emit_outproj_block(a2p, r_prev, rb):
                row0 = r_prev * RC + rb * 128
                osb = p_o.tile([128, DLLM], F32, tag="osb")
                for ncc in range(8):
                    nsl = slice(ncc * 512, (ncc + 1) * 512)
                    ops = psO.tile([128, 512], F32, tag="ops")
                    chains = [("t", 0), ("t", 1), ("d", 0), ("d", 1)]
                    for kk, (bb, mcc) in enumerate(chains):
                        nc.tensor.matmul(
                            ops[:], a2p[bb][mcc][:, rb * 128:(rb + 1) * 128],
                            wo_t[:, kk, nsl],
                            start=(kk == 0), stop=(kk == 3 and not with_bo))
                    if with_bo:
                        nc.tensor.matmul(
                            ops[:], g2_sb[:, row0:row0 + 128], bo_sb[:, nsl],
                            start=False, stop=True)
                    nc.vector.tensor_copy(osb[:, nsl], ops[:])
                nc.sync.dma_start(out.ap()[row0:row0 + 128, :], osb[:])

            pending = None   # (a2 dict, r) awaiting output projection
            for r in range(NRC):
                rsl = slice(r * RC, (r + 1) * RC)
                a2 = {b: [p_a.tile([128, RC], BF16, tag=f"a2_{b}{mc}",
                                   name=f"a2_{b}{mc}")
                          for mc in range(2)] for b in "td"}
                units = [(b, mc) for b in "td" for mc in range(2)]
                prev = None   # (aps, b, mc, p2dict)
                for u, (b, mc) in enumerate(units):
                    aps = [psPV.tile([65, RC], F32, tag=f"aps{mc}{hh}",
                                     name=f"aps{mc}{hh}") for hh in range(2)]
                    p2buf = {}
                    for si in range(NSC):
                        p2buf[si] = emit_qk_exp(b, mc, si, rsl)
                        if prev is not None:
                            paps, pb, pmc, pp2 = prev
                            emit_pv(paps, pb, pmc, si, pp2[si])
                    if prev is not None:
                        emit_norm(prev[0], prev[1], prev[2], a2, rsl)
                    # dense independent PE work between attention units
                    if pending is not None:
                        emit_outproj_block(pending[0], pending[1], u)
                    prev = (aps, b, mc, p2buf)
                # drain last unit of this rows-chunk
                paps, pb, pmc, pp2 = prev
                for si in range(NSC):
                    emit_pv(paps, pb, pmc, si, pp2[si])
                emit_norm(paps, pb, pmc, a2, rsl)
                pending = (a2, r)
            for rb in range(4):
                emit_outproj_block(pending[0], pending[1], rb)

    nc.compile()
    return nc


def _prep_inputs(inputs):
    """Host-side shard + transpose. Returns in_maps for 8 cores."""
    f32 = np.float32
    t = {k: np.asarray(v) for k, v in inputs.items()}
    x_full = {"t": t["trend_emb"].reshape(B * L, D).astype(f32),
              "d": t["detail_emb"].reshape(B * L, D).astype(f32)}
    pT_full = {"t": np.ascontiguousarray(t["trend_proto"].astype(f32).T).astype(bf16),
               "d": np.ascontiguousarray(t["detail_proto"].astype(f32).T).astype(bf16)}
    W = {("q", "t"): t["t_Wq"], ("q", "d"): t["d_Wq"],
         ("k", "t"): t["t_Wk"], ("k", "d"): t["d_Wk"],
         ("v", "t"): t["t_Wv"], ("v", "d"): t["d_Wv"],
         ("o", "t"): t["t_Wo"], ("o", "d"): t["d_Wo"]}
    bias = {("q", "t"): t["t_bq"], ("q", "d"): t["d_bq"],
            ("k", "t"): t["t_bk"], ("k", "d"): t["d_bk"],
            ("v", "t"): t["t_bv"], ("v", "d"): t["d_bv"],
            ("o", "t"): t["t_bo"], ("o", "d"): t["d_bo"]}

    with_bo = bool(np.any(bias[("o", "t")]) or np.any(bias[("o", "d")]))
    in_maps = []
    for core in range(8):
        rg, hg = divmod(core, HG)
        rows = slice(rg * R, (rg + 1) * R)
        hsl = slice(hg * HEC, (hg + 1) * HEC)
        m = {}
        for b in "td":
            m[f"xT_{b}"] = np.ascontiguousarray(x_full[b][rows].T).astype(bf16)
            m[f"pT_{b}"] = pT_full[b]
            m[f"wq_{b}"] = np.ascontiguousarray(W[("q", b)][:, hsl]).astype(bf16)
            m[f"wk_{b}"] = np.ascontiguousarray(W[("k", b)][:, hsl]).astype(bf16)
            m[f"wv_{b}"] = np.ascontiguousarray(W[("v", b)][:, hsl]).astype(bf16)
        m["wo"] = np.vstack([W[("o", "t")][hsl, :], W[("o", "d")][hsl, :]]).astype(bf16)
        m["w1"] = t["g_W1"].astype(bf16)
        m["w2"] = t["g_W2"].astype(bf16)
        m["bq2"] = np.stack([bias[("q", "t")][hsl][0:128], bias[("q", "t")][hsl][128:256],
                             bias[("q", "d")][hsl][0:128], bias[("q", "d")][hsl][128:256]],
                            axis=1).astype(f32)
        m["bk2"] = np.stack([bias[("k", "t")][hsl][0:128], bias[("k", "t")][hsl][128:256],
                             bias[("k", "d")][hsl][0:128], bias[("k", "d")][hsl][128:256]],
                            axis=1).astype(f32)
        m["bv"] = np.concatenate([bias[("v", "t")][hsl],
                                  bias[("v", "d")][hsl]])[None, :].astype(bf16)
        m["gb1"] = np.ascontiguousarray(
            t["g_b1"].astype(f32).reshape(KD, 128).T)
        m["gb2"] = t["g_b2"].astype(f32).reshape(1, 1)
        m["ones"] = np.ones((1, 2048), f32)
        if with_bo:
            m["bo2"] = (np.stack([bias[("o", "t")], bias[("o", "d")]]) / HG).astype(bf16)
        in_maps.append(m)
    return in_maps, with_bo


def kernel(**inputs):
    global LAST_RESULTS
    import os
    from concourse.bass_utils import run_bass_kernel_spmd

    in_maps, with_bo = _prep_inputs(inputs)
    if with_bo not in _CACHE:
        _CACHE[with_bo] = _build(with_bo)
    nc = _CACHE[with_bo]

    trace = bool(os.environ.get("KERNEL_TRACE"))
    res = run_bass_kernel_spmd(
        nc, in_maps, list(range(8)),
        trace=trace, trace_cores=list(range(8)) if trace else None)
    LAST_RESULTS = res

    out = np.empty((RG, R, DLLM), np.float32)
    for rg in range(RG):
        acc = res.results[rg * HG]["out"].astype(np.float32)
        for hg in range(1, HG):
            acc = acc + res.results[rg * HG + hg]["out"]
        out[rg] = acc
    return out.reshape(B, L, DLLM)
